# revision 4
# baseline (speedup 1.0000x reference)
"""Trainium2 Bass kernel: PreActBlock with DoReFa 4-bit quantization (sync-BN).

  out = conv3x3(q(relu(BN1(conv3x3(q(relu(BN0(x))), qw(w0))))), qw(w1)) + x

Design (8 cores, data-parallel over batch 16 -> 2 images/core):
 - Quantized activations are integers 0..15 and quantized weights odd integers
   -15..15 (x scale).  Both are exact in fp8e4 (e4m3), and the PE accumulates
   in fp32, so every conv is computed EXACTLY as integer sums (|S| < 2^20).
 - BN batch stats are all-reduced across the 8 cores (sync-BN semantics).
 - Spatial layout: per image a zero-padded 58x58 grid, 2 images concatenated
   (116 logical rows) plus one guard row top/bottom -> act tiles [128,118,58].
   Conv = 9 shifted matmuls per ci-chunk accumulated in PSUM over windows of
   8 padded rows (N = 464 <= 512 = one PSUM bank).
 - Rounding uses the fp32 magic constant 1.5*2^23 (exact round-to-nearest-even,
   validated on HW), clip via tensor_scalar max/min.
"""
import os
import sys

sys.path.insert(0, "/opt/trn_rl_repo")

import numpy as np

import concourse.bacc as bacc
import concourse.bass as bass
import concourse.mybir as mybir
from concourse import tile
from concourse import bass_utils

F32 = mybir.dt.float32
FP8 = mybir.dt.float8e4
AX = mybir.AxisListType
OP = mybir.AluOpType
AF = mybir.ActivationFunctionType

P = 128
N_CORES = 8
IMG = 2              # images per core
H = 56
HP = 58              # padded row width
ROWS = 116           # 2 images x 58 padded rows
CNT = 50176.0        # global BN count: 16 * 56 * 56
EPS = 1e-5
C_RND = 12582912.0   # 1.5 * 2**23, fp32 RNE rounding constant

# windows of output rows (logical padded-row index, nrows)
WINDOWS = [(1 + 8 * k, 8) for k in range(14)] + [(113, 2)]
# groups of windows sharing weight loads
GROUPS = [WINDOWS[0:4], WINDOWS[4:8], WINDOWS[8:12], WINDOWS[12:15]]


def _runs(r0, nr):
    """Interior row-runs of a window: (logical_row, nrows, img, h0)."""
    out = []
    for lo, hi, img, base in ((1, 56, 0, 1), (59, 114, 1, 59)):
        a, b = max(r0, lo), min(r0 + nr - 1, hi)
        if a <= b:
            out.append((a, b - a + 1, img, a - base))
    return out


def build():
    nc = bacc.Bacc("TRN2", target_bir_lowering=False, debug=False,
                   enable_asserts=False, num_devices=N_CORES)

    x_d = nc.dram_tensor("x", [IMG, 256, H, H], F32, kind="ExternalInput")
    w_d = [nc.dram_tensor("conv0_w", [256, 256, 3, 3], F32, kind="ExternalInput"),
           nc.dram_tensor("conv1_w", [256, 256, 3, 3], F32, kind="ExternalInput")]
    g_d = [nc.dram_tensor("bn0_gamma", [256], F32, kind="ExternalInput"),
           nc.dram_tensor("bn1_gamma", [256], F32, kind="ExternalInput")]
    b_d = [nc.dram_tensor("bn0_beta", [256], F32, kind="ExternalInput"),
           nc.dram_tensor("bn1_beta", [256], F32, kind="ExternalInput")]
    out_d = nc.dram_tensor("out", [IMG, 256, H, H], F32, kind="ExternalOutput")

    xv = x_d.ap().rearrange("n c h w -> c n h w")       # [256, 2, 56, 56]
    ov = out_d.ap().rearrange("n c h w -> c n h w")

    with tile.TileContext(nc) as tc:
        with tc.tile_pool(name="act", bufs=1) as actp, \
             tc.tile_pool(name="wtp", bufs=1) as wtp, \
             tc.tile_pool(name="wq", bufs=4) as wqp, \
             tc.tile_pool(name="qt", bufs=3) as qtp, \
             tc.tile_pool(name="run", bufs=6) as runp, \
             tc.tile_pool(name="st", bufs=1) as stp, \
             tc.tile_pool(name="ps", bufs=8, space="PSUM") as psp, \
             tc.tile_pool(name="dram", bufs=1, space="DRAM") as drp:

            # ---------- static tiles ----------
            act0 = [actp.tile([P, 118, HP], FP8, name=f"act0_{c}") for c in range(2)]
            act1 = [actp.tile([P, 118, HP], FP8, name=f"act1_{c}") for c in range(2)]
            # weights, quantized codes, [ci, tap, co] fp8
            wT = [[wtp.tile([P, 9, 256], FP8, name=f"w{v}T_{c}") for c in range(2)]
                  for v in range(2)]
            # DRAM spill of conv0 integer outputs (unpadded interior)
            s_dram = [drp.tile([P, IMG, H, H], F32, name=f"s_dram_{c}")
                      for c in range(2)]
            ar_in = [drp.tile([P, 4], F32, name=f"ar_in_{i}") for i in range(2)]
            ar_out = [drp.tile([P, 4], F32, name=f"ar_out_{i}") for i in range(2)]

            # stats / small vectors
            xbn = [stp.tile([P, 16, 6], F32, name=f"xbn_{c}") for c in range(2)]
            sumS = [stp.tile([P, 16], F32, name=f"sumS_{c}") for c in range(2)]
            sqS = [stp.tile([P, 16], F32, name=f"sqS_{c}") for c in range(2)]
            gvec = [[stp.tile([P, 1], F32, name=f"g{v}_{c}") for c in range(2)]
                    for v in range(2)]
            bvec = [[stp.tile([P, 1], F32, name=f"b{v}_{c}") for c in range(2)]
                    for v in range(2)]
            avec = [[stp.tile([P, 1], F32, name=f"a{v}_{c}") for c in range(2)]
                    for v in range(2)]
            bbvec = [[stp.tile([P, 1], F32, name=f"bb{v}_{c}") for c in range(2)]
                     for v in range(2)]
            svec = [stp.tile([P, 1], F32, name=f"scale_{v}") for v in range(2)]
            pk = [stp.tile([P, 4], F32, name=f"pk_{i}") for i in range(2)]
            gpk = [stp.tile([P, 4], F32, name=f"gpk_{i}") for i in range(2)]

            def vtile(name):
                return stp.tile([P, 1], F32, name=name, tag="vtmp", bufs=8)

            # ---------- tiny vector helpers (all on [P,1]) ----------
            def refined_rsqrt(vpe, name):
                """rsqrt(vpe) with 2 Newton refinements (vpe > 0)."""
                r = vtile(f"{name}_r")
                nc.vector.reciprocal(r[:], vpe[:])
                y = vtile(f"{name}_y")
                nc.scalar.activation(y[:], r[:], AF.Sqrt)
                for i in range(2):
                    y2 = vtile(f"{name}_y2{i}")
                    nc.vector.tensor_mul(y2[:], y[:], y[:])
                    t2 = vtile(f"{name}_t2{i}")
                    nc.vector.tensor_mul(t2[:], vpe[:], y2[:])
                    h = vtile(f"{name}_h{i}")
                    nc.vector.tensor_scalar(h[:], t2[:], -0.5, 1.5, OP.mult, OP.add)
                    yn = vtile(f"{name}_yn{i}")
                    nc.vector.tensor_mul(yn[:], y[:], h[:])
                    y = yn
                return y

            def bn_coeffs(v, c, gsum, gsumsq, scale):
                """a,b for z = a*S + b  (= 15 * BN-affine), scale=None for BN0."""
                mean = vtile(f"m{v}{c}")
                nc.vector.tensor_scalar(mean[:], gsum, 1.0 / CNT, None, OP.mult)
                ex2 = vtile(f"e{v}{c}")
                nc.vector.tensor_scalar(ex2[:], gsumsq, 1.0 / CNT, None, OP.mult)
                m2 = vtile(f"m2{v}{c}")
                nc.vector.tensor_mul(m2[:], mean[:], mean[:])
                var = vtile(f"va{v}{c}")
                nc.vector.tensor_sub(var[:], ex2[:], m2[:])
                if scale is not None:
                    s2 = vtile(f"s2{v}{c}")
                    nc.vector.tensor_mul(s2[:], scale[:], scale[:])
                    nc.vector.tensor_mul(var[:], var[:], s2[:])
                    mo = vtile(f"mo{v}{c}")
                    nc.vector.tensor_mul(mo[:], mean[:], scale[:])
                    mean = mo
                vpe = vtile(f"vp{v}{c}")
                nc.vector.tensor_scalar(vpe[:], var[:], EPS, None, OP.add)
                rs = refined_rsqrt(vpe, f"rs{v}{c}")
                grs = vtile(f"gr{v}{c}")
                nc.vector.tensor_mul(grs[:], gvec[v][c][:], rs[:])
                a = avec[v][c]
                if scale is not None:
                    asc = vtile(f"as{v}{c}")
                    nc.vector.tensor_mul(asc[:], grs[:], scale[:])
                    nc.vector.tensor_scalar(a[:], asc[:], 15.0, None, OP.mult)
                else:
                    nc.vector.tensor_scalar(a[:], grs[:], 15.0, None, OP.mult)
                mg = vtile(f"mg{v}{c}")
                nc.vector.tensor_mul(mg[:], mean[:], grs[:])
                mg15 = vtile(f"mh{v}{c}")
                nc.vector.tensor_scalar(mg15[:], mg[:], 15.0, None, OP.mult)
                b15 = vtile(f"bh{v}{c}")
                nc.vector.tensor_scalar(b15[:], bvec[v][c][:], 15.0, None, OP.mult)
                nc.vector.tensor_sub(bbvec[v][c][:], b15[:], mg15[:])

            # ---------- load BN params ----------
            for v in range(2):
                for c in range(2):
                    nc.sync.dma_start(gvec[v][c][:], g_d[v].ap()[c * P:(c + 1) * P])
                    nc.sync.dma_start(bvec[v][c][:], b_d[v].ap()[c * P:(c + 1) * P])

            # ---------- act border zeroing ----------
            with nc.named_scope("memset"):
                for t in act0 + act1:
                    for sl in (np.s_[:, 0:2, :], np.s_[:, 58:60, :],
                               np.s_[:, 116:118, :], np.s_[:, :, 0:1],
                               np.s_[:, :, 57:58]):
                        nc.gpsimd.memset(t[sl], 0.0)

            # ---------- weight quantization (both convs) ----------
            # layout in DRAM: [o, i, kh, kw]; we DMA strided to [ci, co, tap]
            with nc.named_scope("wquant"):
                mxp = [stp.tile([P, 4], F32, name=f"mxp_{v}") for v in range(2)]
                wnat = {}
                for v in range(2):
                    wv = w_d[v].ap().rearrange("o i kh kw -> i o (kh kw)")
                    for c in range(2):
                        for hh in range(2):
                            wn = wqp.tile([P, P, 9], F32, name=f"wn{v}{c}{hh}",
                                          tag="wnat", bufs=4)
                            nc.sync.dma_start(
                                wn[:], wv[c * P:(c + 1) * P,
                                          hh * P:(hh + 1) * P, :])
                            wnat[(v, c, hh)] = wn
                for v in range(2):
                    for i, (c, hh) in enumerate(((0, 0), (0, 1), (1, 0), (1, 1))):
                        t = wqp.tile([P, P, 9], F32, name=f"t{v}{c}{hh}",
                                     tag=f"tanh{v}", bufs=4)
                        nc.scalar.activation(t[:], wnat[(v, c, hh)][:], AF.Tanh)
                        wnat[(v, c, hh)] = None
                        nc.vector.tensor_reduce(
                            mxp[v][:, i:i + 1], t[:], AX.XY, OP.max,
                            apply_absolute_value=True)
                        wnat[(v, c, hh, "t")] = t
                    mx1 = vtile(f"mx1_{v}")
                    nc.vector.tensor_reduce(mx1[:], mxp[v][:], AX.X, OP.max,
                                            apply_absolute_value=True)
                    msc = stp.tile([1, 1], F32, name=f"msc_{v}")
                    nc.gpsimd.tensor_reduce(msc[:], mx1[:], AX.C, OP.max)
                    mvec = vtile(f"mvec_{v}")
                    nc.gpsimd.partition_broadcast(mvec[:], msc[:])
                    # svec = M/225 (psum scale); rinv = 7.5/M for codes
                    nc.vector.tensor_scalar(svec[v][:], mvec[:], 1.0 / 225.0,
                                            None, OP.mult)
                    r = vtile(f"rin_{v}")
                    nc.vector.reciprocal(r[:], mvec[:])
                    for i in range(2):  # Newton: r = r*(2 - M*r)
                        t1 = vtile(f"rn1_{v}{i}")
                        nc.vector.tensor_mul(t1[:], mvec[:], r[:])
                        t2 = vtile(f"rn2_{v}{i}")
                        nc.vector.tensor_scalar(t2[:], t1[:], -1.0, 2.0,
                                                OP.mult, OP.add)
                        rn = vtile(f"rn3_{v}{i}")
                        nc.vector.tensor_mul(rn[:], r[:], t2[:])
                        r = rn
                    sc = vtile(f"sc_{v}")
                    nc.vector.tensor_scalar(sc[:], r[:], 7.5, None, OP.mult)
                    for i, (c, hh) in enumerate(((0, 0), (0, 1), (1, 0), (1, 1))):
                        eng = nc.vector if i % 2 == 0 else nc.gpsimd
                        t = wnat[(v, c, hh, "t")]
                        z = wqp.tile([P, P, 9], F32, name=f"z{v}{c}{hh}",
                                     tag="wz", bufs=4)
                        eng.tensor_scalar(z[:], t[:], sc[:], 7.5, OP.mult, OP.add)
                        u = wqp.tile([P, P, 9], F32, name=f"u{v}{c}{hh}",
                                     tag="wu", bufs=4)
                        eng.tensor_scalar(u[:], z[:], C_RND, None, OP.add)
                        vv = wqp.tile([P, P, 9], F32, name=f"v{v}{c}{hh}",
                                      tag="wv", bufs=4)
                        eng.tensor_scalar(vv[:], u[:], -C_RND, 2.0, OP.add, OP.mult)
                        # write codes 2r-15 (odd ints) into [ci, tap, co] fp8
                        dst = wT[v][c][:, :, hh * P:(hh + 1) * P]
                        eng.tensor_scalar(dst.rearrange("p t c -> p c t"),
                                          vv[:], -15.0, None, OP.add)

            # ---------- BN0 stats over x (streamed) ----------
            with nc.named_scope("stats0"):
                for c in range(2):
                    for img in range(IMG):
                        for blk in range(4):
                            h0 = blk * 14
                            xt = qtp.tile([P, 14, H], F32, name=f"sx{c}{img}{blk}",
                                          tag="qin", bufs=4)
                            nc.sync.dma_start(
                                xt[:], xv[c * P:(c + 1) * P, img, h0:h0 + 14, :])
                            fl = xt.rearrange("p a b -> p (a b)")
                            k = (img * 4 + blk) * 2
                            nc.vector.bn_stats(xbn[c][:, k, :], fl[:, 0:392])
                            nc.vector.bn_stats(xbn[c][:, k + 1, :], fl[:, 392:784])
                    mv = stp.tile([P, 2], F32, name=f"mv0_{c}")
                    nc.vector.bn_aggr(mv[:], xbn[c][:, 0:16, :])
                    # local sums: n*(mean, var+mean^2), n = 6272
                    nc.vector.tensor_scalar(pk[0][:, 2 * c:2 * c + 1],
                                            mv[:, 0:1], 6272.0, None, OP.mult)
                    m2 = vtile(f"xm2_{c}")
                    nc.vector.tensor_mul(m2[:], mv[:, 0:1], mv[:, 0:1])
                    vp = vtile(f"xvp_{c}")
                    nc.vector.tensor_add(vp[:], mv[:, 1:2], m2[:])
                    nc.vector.tensor_scalar(pk[0][:, 2 * c + 1:2 * c + 2],
                                            vp[:], 6272.0, None, OP.mult)
                nc.sync.dma_start(ar_in[0][:], pk[0][:])
                nc.gpsimd.collective_compute(
                    "AllReduce", OP.add, replica_groups=[list(range(N_CORES))],
                    ins=[ar_in[0].opt()], outs=[ar_out[0].opt()])
                nc.sync.dma_start(gpk[0][:], ar_out[0][:])
                for c in range(2):
                    bn_coeffs(0, c, gpk[0][:, 2 * c:2 * c + 1],
                              gpk[0][:, 2 * c + 1:2 * c + 2], None)

            # ---------- quantize0: x -> act0 codes (streamed again) ----------
            def quantize_block(src_ap, dst_ap, a, b, eng, names):
                z = qtp.tile([P, 14, H], F32, name=names + "z", tag="qz", bufs=3)
                nc.scalar.activation(z[:], src_ap, AF.Identity,
                                     bias=b[:], scale=a[:])
                u = qtp.tile([P, 14, H], F32, name=names + "u", tag="qu", bufs=3)
                eng.tensor_scalar(u[:], z[:], C_RND, C_RND, OP.add, OP.max)
                eng.tensor_scalar(dst_ap, u[:], -C_RND, 15.0, OP.add, OP.min)

            with nc.named_scope("quant0"):
                for c in range(2):
                    for img in range(IMG):
                        for blk in range(4):
                            h0 = blk * 14
                            xt = qtp.tile([P, 14, H], F32, name=f"qx{c}{img}{blk}",
                                          tag="qin", bufs=4)
                            nc.sync.dma_start(
                                xt[:], xv[c * P:(c + 1) * P, img, h0:h0 + 14, :])
                            pr = img * 58 + 2 + h0   # physical row in act tile
                            dst = act0[c][:, pr:pr + 14, 1:57]
                            eng = nc.vector if (img * 4 + blk) % 2 == 0 else nc.gpsimd
                            quantize_block(xt[:], dst, avec[0][c], bbvec[0][c],
                                           eng, f"q0_{c}{img}{blk}")

            # ---------- conv (shared) ----------
            def conv(v, acts, epilogue):
                flats = [acts[c].rearrange("p r c -> p (r c)") for c in range(2)]
                for gi, grp in enumerate(GROUPS):
                    for co in range(2):
                        psums = []
                        for wi, (r0, nr) in enumerate(grp):
                            ps = psp.tile([P, nr, HP], F32,
                                          name=f"ps{v}_{gi}_{co}_{wi}",
                                          tag="psw", padded_shape=[P, 8, HP])
                            psums.append(ps)
                        for tap in range(9):
                            dy, dx = divmod(tap, 3)
                            off = (dy - 1) * HP + (dx - 1)
                            for ci in range(2):
                                lhsT = wT[v][ci][:, tap, co * P:(co + 1) * P]
                                first = (tap == 0 and ci == 0)
                                last = (tap == 8 and ci == 1)
                                for wi, (r0, nr) in enumerate(grp):
                                    st = (r0 + 1) * HP + off
                                    rhs = flats[ci][:, st:st + nr * HP]
                                    nc.tensor.matmul(
                                        psums[wi].rearrange("p r c -> p (r c)"),
                                        lhsT, rhs, start=first, stop=last)
                        for wi, (r0, nr) in enumerate(grp):
                            epilogue(co, r0, nr, psums[wi])

            # ---------- conv0 + epilogue (spill S + interior sums) ----------
            slot_idx = [0, 0]

            def epi0(co, r0, nr, ps):
                for (rl, n, img, h0) in _runs(r0, nr):
                    sl = ps[:, rl - r0:rl - r0 + n, 1:57]
                    tmp = runp.tile([P, n, H], F32, name=f"s_{co}_{rl}",
                                    tag="srun", bufs=6, padded_shape=[P, 8, H])
                    k = slot_idx[co]
                    slot_idx[co] += 1
                    nc.scalar.activation(tmp[:], sl, AF.Identity,
                                         accum_out=sumS[co][:, k:k + 1])
                    sq = runp.tile([P, n, H], F32, name=f"sq_{co}_{rl}",
                                   tag="sq", bufs=4, padded_shape=[P, 8, H])
                    nc.vector.scalar_tensor_tensor(
                        sq[:], tmp[:], 0.0, tmp[:], OP.bypass, OP.mult,
                        accum_out=sqS[co][:, k:k + 1])
                    nc.sync.dma_start(s_dram[co][:, img, h0:h0 + n, :], tmp[:])

            with nc.named_scope("conv0"):
                conv(0, act0, epi0)

            # ---------- BN1 stats + AR + coeffs ----------
            with nc.named_scope("stats1"):
                for c in range(2):
                    ns = slot_idx[c]
                    nc.vector.tensor_reduce(pk[1][:, 2 * c:2 * c + 1],
                                            sumS[c][:, 0:ns], AX.X, OP.add)
                    nc.vector.tensor_reduce(pk[1][:, 2 * c + 1:2 * c + 2],
                                            sqS[c][:, 0:ns], AX.X, OP.add)
                nc.sync.dma_start(ar_in[1][:], pk[1][:])
                nc.gpsimd.collective_compute(
                    "AllReduce", OP.add, replica_groups=[list(range(N_CORES))],
                    ins=[ar_in[1].opt()], outs=[ar_out[1].opt()])
                nc.sync.dma_start(gpk[1][:], ar_out[1][:])
                for c in range(2):
                    bn_coeffs(1, c, gpk[1][:, 2 * c:2 * c + 1],
                              gpk[1][:, 2 * c + 1:2 * c + 2], svec[0])

            # ---------- quantize1: S -> act1 codes ----------
            with nc.named_scope("quant1"):
                for c in range(2):
                    for img in range(IMG):
                        for blk in range(4):
                            h0 = blk * 14
                            st = qtp.tile([P, 14, H], F32, name=f"qs{c}{img}{blk}",
                                          tag="qin", bufs=4)
                            nc.sync.dma_start(
                                st[:], s_dram[c][:, img, h0:h0 + 14, :])
                            pr = img * 58 + 2 + h0
                            dst = act1[c][:, pr:pr + 14, 1:57]
                            eng = nc.vector if (img * 4 + blk) % 2 == 0 else nc.gpsimd
                            quantize_block(st[:], dst, avec[1][c], bbvec[1][c],
                                           eng, f"q1_{c}{img}{blk}")

            # ---------- conv1 + residual epilogue ----------
            def epi1(co, r0, nr, ps):
                for (rl, n, img, h0) in _runs(r0, nr):
                    sl = ps[:, rl - r0:rl - r0 + n, 1:57]
                    xt = runp.tile([P, n, H], F32, name=f"x_{co}_{rl}",
                                   tag="xrun", bufs=6, padded_shape=[P, 8, H])
                    nc.sync.dma_start(
                        xt[:], xv[co * P:(co + 1) * P, img, h0:h0 + n, :])
                    ot = runp.tile([P, n, H], F32, name=f"o_{co}_{rl}",
                                   tag="srun", bufs=6, padded_shape=[P, 8, H])
                    nc.vector.scalar_tensor_tensor(
                        ot[:], sl, svec[1][:], xt[:], OP.mult, OP.add)
                    nc.sync.dma_start(
                        ov[co * P:(co + 1) * P, img, h0:h0 + n, :], ot[:])

            with nc.named_scope("conv1"):
                conv(1, act1, epi1)

    nc.compile()
    return nc


def _install_ntff_hook():
    """Provide antenv.axon_hooks (absent in this image) via ctypes so that
    run_bass_kernel_spmd(trace=True) can capture NTFF profiles."""
    try:
        from antenv.axon_hooks import get_axon_ntff_profile_hook  # noqa: F401
        return
    except ImportError:
        pass
    import contextlib
    import ctypes
    import types

    so_path = "/opt/axon/libaxon_pjrt.so"
    if not os.path.exists(so_path):
        return
    lib = ctypes.CDLL(so_path)
    if not hasattr(lib, "axon_start_nrt_profile"):
        return
    lib.axon_start_nrt_profile.argtypes = [ctypes.POINTER(ctypes.c_int64),
                                           ctypes.c_size_t]
    lib.axon_start_nrt_profile.restype = ctypes.c_int64
    lib.axon_stop_nrt_profile.argtypes = [ctypes.c_char_p]
    lib.axon_stop_nrt_profile.restype = ctypes.c_int64

    @contextlib.contextmanager
    def _hook(output_dir, device_ids):
        import jax
        jax.devices()
        if device_ids:
            ids = (ctypes.c_int64 * len(device_ids))(*device_ids)
            rc = lib.axon_start_nrt_profile(ids, len(device_ids))
        else:
            rc = lib.axon_start_nrt_profile(None, 0)
        if rc != 0:
            raise RuntimeError(f"axon_start_nrt_profile rc={rc}")
        try:
            yield
        finally:
            n = lib.axon_stop_nrt_profile(str(output_dir).encode())
            print(f"ntff profile: {n} file(s) written to {output_dir}")

    hook_holder = [_hook]
    mod = types.ModuleType("antenv.axon_hooks")
    mod.get_axon_ntff_profile_hook = lambda: hook_holder[0]
    mod.set_axon_ntff_profile_hook = lambda h: hook_holder.__setitem__(0, h)
    import antenv
    sys.modules["antenv.axon_hooks"] = mod
    antenv.axon_hooks = mod


_NC = None


def _get_nc():
    global _NC
    if _NC is None:
        _NC = build()
    return _NC


LAST_RESULTS = None


def kernel(x, bn0_gamma, bn0_beta, conv0_w, bn1_gamma, bn1_beta, conv1_w):
    global LAST_RESULTS
    nc = _get_nc()
    shared = {
        "conv0_w": np.ascontiguousarray(conv0_w, np.float32),
        "conv1_w": np.ascontiguousarray(conv1_w, np.float32),
        "bn0_gamma": np.ascontiguousarray(bn0_gamma, np.float32),
        "bn0_beta": np.ascontiguousarray(bn0_beta, np.float32),
        "bn1_gamma": np.ascontiguousarray(bn1_gamma, np.float32),
        "bn1_beta": np.ascontiguousarray(bn1_beta, np.float32),
    }
    x = np.ascontiguousarray(x, np.float32)
    in_maps = [{"x": x[2 * c:2 * c + 2], **shared} for c in range(N_CORES)]
    trace = bool(int(os.environ.get("KERNEL_TRACE", "0")))
    if trace:
        _install_ntff_hook()
    res = bass_utils.run_bass_kernel_spmd(
        nc, in_maps, core_ids=list(range(N_CORES)), trace=trace)
    LAST_RESULTS = res
    return np.concatenate([res.results[c]["out"] for c in range(N_CORES)], axis=0)


# revision 5
# speedup vs baseline: 1.3459x; 1.3459x over previous
"""Trainium2 Bass kernel: PreActBlock with DoReFa 4-bit quantization (sync-BN).

  out = conv3x3(q(relu(BN1(conv3x3(q(relu(BN0(x))), qw(w0))))), qw(w1)) + x

Design (8 cores, data-parallel over batch 16 -> 2 images/core):
 - Quantized activations are integers 0..15 and quantized weights odd integers
   -15..15 (x scale).  Both are exact in fp8e4 (e4m3) and the PE accumulates
   in fp32, so every conv is computed EXACTLY as integer sums (|S| < 2^20).
 - BN batch stats are all-reduced across the 8 cores (sync-BN semantics).
 - Rounding: fp32->int8 cast is round-to-nearest-even (HW-validated), so
   quantize = clip(tensor_scalar max/min -> int8) + int8->fp8 copy.
 - Spatial layout: unpadded 56-wide rows; 2 images stacked with zero pad rows
   (116 rows).  Column zero-padding is implicit: edge taps run one column
   narrower (the skipped outputs receive exactly the zero-pad contribution).
 - Conv: per 9-row window (N = 504 <= 512, one PSUM bank) accumulate
   9 taps x 2 ci-chunks of shifted matmuls; weights host-permuted to
   [i, kh, kw, o] so weight DMA and quantized code writes are contiguous.
"""
import os
import sys

sys.path.insert(0, "/opt/trn_rl_repo")

import numpy as np

import concourse.bacc as bacc
import concourse.bass as bass
import concourse.mybir as mybir
from concourse import tile
from concourse import bass_utils

F32 = mybir.dt.float32
FP8 = mybir.dt.float8e4
I8 = mybir.dt.int8
AX = mybir.AxisListType
OP = mybir.AluOpType
AF = mybir.ActivationFunctionType

P = 128
N_CORES = 8
IMG = 2              # images per core
H = 56
ROWS = 116           # 2 images x (1 pad + 56 + 1 pad) rows
CNT = 50176.0        # global BN count: 16 * 56 * 56
EPS = 1e-5

# 9-row output windows over logical rows 1..114
WINDOWS = [(1 + 9 * k, 9) for k in range(12)] + [(109, 6)]
GROUPS = [WINDOWS[0:4], WINDOWS[4:8], WINDOWS[8:12], WINDOWS[12:13]]
# tap order: full-width tap (dy=0,dx=1) first so start=True covers all columns
TAPS = [(0, 1), (0, 0), (0, 2), (1, 0), (1, 1), (1, 2), (2, 0), (2, 1), (2, 2)]


def _runs(r0, nr):
    """Interior row-runs of a window: (logical_row, nrows, img, h0)."""
    out = []
    for lo, hi, img, base in ((1, 56, 0, 1), (59, 114, 1, 59)):
        a, b = max(r0, lo), min(r0 + nr - 1, hi)
        if a <= b:
            out.append((a, b - a + 1, img, a - base))
    return out


def build():
    nc = bacc.Bacc("TRN2", target_bir_lowering=False, debug=False,
                   enable_asserts=False, num_devices=N_CORES)

    x_d = nc.dram_tensor("x", [IMG, 256, H, H], F32, kind="ExternalInput")
    # host-permuted to [i, kh, kw, o]
    w_d = [nc.dram_tensor("conv0_w", [256, 3, 3, 256], F32, kind="ExternalInput"),
           nc.dram_tensor("conv1_w", [256, 3, 3, 256], F32, kind="ExternalInput")]
    g_d = [nc.dram_tensor("bn0_gamma", [256], F32, kind="ExternalInput"),
           nc.dram_tensor("bn1_gamma", [256], F32, kind="ExternalInput")]
    b_d = [nc.dram_tensor("bn0_beta", [256], F32, kind="ExternalInput"),
           nc.dram_tensor("bn1_beta", [256], F32, kind="ExternalInput")]
    out_d = nc.dram_tensor("out", [IMG, 256, H, H], F32, kind="ExternalOutput")

    xv = x_d.ap().rearrange("n c h w -> c n h w")       # [256, 2, 56, 56]
    ov = out_d.ap().rearrange("n c h w -> c n h w")

    with tile.TileContext(nc) as tc:
        with tc.tile_pool(name="act", bufs=1) as actp, \
             tc.tile_pool(name="wtp", bufs=1) as wtp, \
             tc.tile_pool(name="wq", bufs=4) as wqp, \
             tc.tile_pool(name="qt", bufs=3) as qtp, \
             tc.tile_pool(name="run", bufs=6) as runp, \
             tc.tile_pool(name="st", bufs=1) as stp, \
             tc.tile_pool(name="ps", bufs=8, space="PSUM") as psp, \
             tc.tile_pool(name="dram", bufs=1, space="DRAM") as drp:

            # ---------- static tiles ----------
            act0 = [actp.tile([P, ROWS, H], FP8, name=f"act0_{c}") for c in range(2)]
            act1 = [actp.tile([P, ROWS, H], FP8, name=f"act1_{c}") for c in range(2)]
            # quantized weight codes, [ci, tap, co] fp8
            wT = [[wtp.tile([P, 9, 256], FP8, name=f"w{v}T_{c}") for c in range(2)]
                  for v in range(2)]
            # DRAM spill of conv0 integer outputs (unpadded interior)
            s_dram = [drp.tile([P, IMG, H, H], F32, name=f"s_dram_{c}")
                      for c in range(2)]
            ar_in = [drp.tile([P, 4], F32, name=f"ar_in_{i}") for i in range(2)]
            ar_out = [drp.tile([P, 4], F32, name=f"ar_out_{i}") for i in range(2)]

            # stats / small vectors
            xbn = [stp.tile([P, 16, 6], F32, name=f"xbn_{c}") for c in range(2)]
            sumS = [stp.tile([P, 16], F32, name=f"sumS_{c}") for c in range(2)]
            sqS = [stp.tile([P, 16], F32, name=f"sqS_{c}") for c in range(2)]
            gvec = [[stp.tile([P, 1], F32, name=f"g{v}_{c}") for c in range(2)]
                    for v in range(2)]
            bvec = [[stp.tile([P, 1], F32, name=f"b{v}_{c}") for c in range(2)]
                    for v in range(2)]
            avec = [[stp.tile([P, 1], F32, name=f"a{v}_{c}") for c in range(2)]
                    for v in range(2)]
            bbvec = [[stp.tile([P, 1], F32, name=f"bb{v}_{c}") for c in range(2)]
                     for v in range(2)]
            svec = [stp.tile([P, 1], F32, name=f"scale_{v}") for v in range(2)]
            pk = [stp.tile([P, 4], F32, name=f"pk_{i}") for i in range(2)]
            gpk = [stp.tile([P, 4], F32, name=f"gpk_{i}") for i in range(2)]

            def vtile(name):
                return stp.tile([P, 1], F32, name=name, tag="vtmp", bufs=8)

            # ---------- tiny vector helpers (all on [P,1]) ----------
            def refined_rsqrt(vpe, name):
                """rsqrt(vpe) with 2 Newton refinements (vpe > 0)."""
                r = vtile(f"{name}_r")
                nc.vector.reciprocal(r[:], vpe[:])
                y = vtile(f"{name}_y")
                nc.scalar.activation(y[:], r[:], AF.Sqrt)
                for i in range(2):
                    y2 = vtile(f"{name}_y2{i}")
                    nc.vector.tensor_mul(y2[:], y[:], y[:])
                    t2 = vtile(f"{name}_t2{i}")
                    nc.vector.tensor_mul(t2[:], vpe[:], y2[:])
                    h = vtile(f"{name}_h{i}")
                    nc.vector.tensor_scalar(h[:], t2[:], -0.5, 1.5, OP.mult, OP.add)
                    yn = vtile(f"{name}_yn{i}")
                    nc.vector.tensor_mul(yn[:], y[:], h[:])
                    y = yn
                return y

            def bn_coeffs(v, c, gsum, gsumsq, scale):
                """a,b for z = a*S + b  (= 15 * BN-affine), scale=None for BN0."""
                mean = vtile(f"m{v}{c}")
                nc.vector.tensor_scalar(mean[:], gsum, 1.0 / CNT, None, OP.mult)
                ex2 = vtile(f"e{v}{c}")
                nc.vector.tensor_scalar(ex2[:], gsumsq, 1.0 / CNT, None, OP.mult)
                m2 = vtile(f"m2{v}{c}")
                nc.vector.tensor_mul(m2[:], mean[:], mean[:])
                var = vtile(f"va{v}{c}")
                nc.vector.tensor_sub(var[:], ex2[:], m2[:])
                if scale is not None:
                    s2 = vtile(f"s2{v}{c}")
                    nc.vector.tensor_mul(s2[:], scale[:], scale[:])
                    nc.vector.tensor_mul(var[:], var[:], s2[:])
                    mo = vtile(f"mo{v}{c}")
                    nc.vector.tensor_mul(mo[:], mean[:], scale[:])
                    mean = mo
                vpe = vtile(f"vp{v}{c}")
                nc.vector.tensor_scalar(vpe[:], var[:], EPS, None, OP.add)
                rs = refined_rsqrt(vpe, f"rs{v}{c}")
                grs = vtile(f"gr{v}{c}")
                nc.vector.tensor_mul(grs[:], gvec[v][c][:], rs[:])
                a = avec[v][c]
                if scale is not None:
                    asc = vtile(f"as{v}{c}")
                    nc.vector.tensor_mul(asc[:], grs[:], scale[:])
                    nc.vector.tensor_scalar(a[:], asc[:], 15.0, None, OP.mult)
                else:
                    nc.vector.tensor_scalar(a[:], grs[:], 15.0, None, OP.mult)
                mg = vtile(f"mg{v}{c}")
                nc.vector.tensor_mul(mg[:], mean[:], grs[:])
                mg15 = vtile(f"mh{v}{c}")
                nc.vector.tensor_scalar(mg15[:], mg[:], 15.0, None, OP.mult)
                b15 = vtile(f"bh{v}{c}")
                nc.vector.tensor_scalar(b15[:], bvec[v][c][:], 15.0, None, OP.mult)
                nc.vector.tensor_sub(bbvec[v][c][:], b15[:], mg15[:])

            # ---------- load BN params ----------
            for v in range(2):
                for c in range(2):
                    nc.sync.dma_start(gvec[v][c][:], g_d[v].ap()[c * P:(c + 1) * P])
                    nc.sync.dma_start(bvec[v][c][:], b_d[v].ap()[c * P:(c + 1) * P])

            # ---------- act pad-row zeroing ----------
            with nc.named_scope("memset"):
                for t in act0 + act1:
                    for r in (0, 57, 58, 115):
                        nc.gpsimd.memset(t[:, r, :], 0.0)

            # ---------- weight quantization (both convs) ----------
            # DRAM layout [i, kh, kw, o] -> contiguous [ci, tap, co] tiles
            with nc.named_scope("wquant"):
                mxp = [stp.tile([P, 4], F32, name=f"mxp_{v}") for v in range(2)]
                wnat = {}
                for v in range(2):
                    wv = w_d[v].ap().rearrange("i kh kw o -> i (kh kw) o")
                    for c in range(2):
                        for hh in range(2):  # tap halves: 0 -> taps 0..3, 1 -> 4..8
                            t0, t1 = (0, 4) if hh == 0 else (4, 9)
                            wn = wqp.tile([P, t1 - t0, 256], F32,
                                          name=f"wn{v}{c}{hh}", tag="wnat", bufs=4,
                                          padded_shape=[P, 5, 256])
                            nc.sync.dma_start(
                                wn[:], wv[c * P:(c + 1) * P, t0:t1, :])
                            wnat[(v, c, hh)] = wn
                for v in range(2):
                    for i, (c, hh) in enumerate(((0, 0), (0, 1), (1, 0), (1, 1))):
                        wn = wnat[(v, c, hh)]
                        t = wqp.tile(list(wn.shape), F32, name=f"t{v}{c}{hh}",
                                     tag=f"tanh{v}", bufs=4,
                                     padded_shape=[P, 5, 256])
                        nc.scalar.activation(t[:], wn[:], AF.Tanh)
                        nc.vector.tensor_reduce(
                            mxp[v][:, i:i + 1], t[:], AX.XY, OP.max,
                            apply_absolute_value=True)
                        wnat[(v, c, hh, "t")] = t
                    mx1 = vtile(f"mx1_{v}")
                    nc.vector.tensor_reduce(mx1[:], mxp[v][:], AX.X, OP.max,
                                            apply_absolute_value=True)
                    msc = stp.tile([1, 1], F32, name=f"msc_{v}")
                    nc.gpsimd.tensor_reduce(msc[:], mx1[:], AX.C, OP.max)
                    mvec = vtile(f"mvec_{v}")
                    nc.gpsimd.partition_broadcast(mvec[:], msc[:])
                    # svec = M/225 (psum scale); r = 7.5/M for codes
                    nc.vector.tensor_scalar(svec[v][:], mvec[:], 1.0 / 225.0,
                                            None, OP.mult)
                    r = vtile(f"rin_{v}")
                    nc.vector.reciprocal(r[:], mvec[:])
                    for i in range(2):  # Newton: r = r*(2 - M*r)
                        t1 = vtile(f"rn1_{v}{i}")
                        nc.vector.tensor_mul(t1[:], mvec[:], r[:])
                        t2 = vtile(f"rn2_{v}{i}")
                        nc.vector.tensor_scalar(t2[:], t1[:], -1.0, 2.0,
                                                OP.mult, OP.add)
                        rn = vtile(f"rn3_{v}{i}")
                        nc.vector.tensor_mul(rn[:], r[:], t2[:])
                        r = rn
                    sc = vtile(f"sc_{v}")
                    nc.vector.tensor_scalar(sc[:], r[:], 7.5, None, OP.mult)
                    for i, (c, hh) in enumerate(((0, 0), (0, 1), (1, 0), (1, 1))):
                        eng = nc.vector if i % 2 == 0 else nc.gpsimd
                        t = wnat[(v, c, hh, "t")]
                        sh = list(t.shape)
                        z = wqp.tile(sh, F32, name=f"z{v}{c}{hh}", tag="wz",
                                     bufs=4, padded_shape=[P, 5, 256])
                        eng.tensor_scalar(z[:], t[:], sc[:], 7.5, OP.mult, OP.add)
                        ri = wqp.tile(sh, I8, name=f"ri{v}{c}{hh}", tag="wr",
                                      bufs=4, padded_shape=[P, 5, 256])
                        eng.tensor_scalar(ri[:], z[:], 0.0, 15.0, OP.max, OP.min)
                        t0 = 0 if hh == 0 else 4
                        dst = wT[v][c][:, t0:t0 + sh[1], :]
                        eng.tensor_scalar(dst, ri[:], 2.0, -15.0, OP.mult, OP.add)

            # ---------- BN0 stats over x (streamed) ----------
            with nc.named_scope("stats0"):
                for c in range(2):
                    for img in range(IMG):
                        for blk in range(4):
                            h0 = blk * 14
                            xt = qtp.tile([P, 14, H], F32, name=f"sx{c}{img}{blk}",
                                          tag="qin", bufs=4)
                            nc.sync.dma_start(
                                xt[:], xv[c * P:(c + 1) * P, img, h0:h0 + 14, :])
                            fl = xt.rearrange("p a b -> p (a b)")
                            k = (img * 4 + blk) * 2
                            nc.vector.bn_stats(xbn[c][:, k, :], fl[:, 0:392])
                            nc.vector.bn_stats(xbn[c][:, k + 1, :], fl[:, 392:784])
                    mv = stp.tile([P, 2], F32, name=f"mv0_{c}")
                    nc.vector.bn_aggr(mv[:], xbn[c][:, 0:16, :])
                    # local sums: n*(mean, var+mean^2), n = 6272
                    nc.vector.tensor_scalar(pk[0][:, 2 * c:2 * c + 1],
                                            mv[:, 0:1], 6272.0, None, OP.mult)
                    m2 = vtile(f"xm2_{c}")
                    nc.vector.tensor_mul(m2[:], mv[:, 0:1], mv[:, 0:1])
                    vp = vtile(f"xvp_{c}")
                    nc.vector.tensor_add(vp[:], mv[:, 1:2], m2[:])
                    nc.vector.tensor_scalar(pk[0][:, 2 * c + 1:2 * c + 2],
                                            vp[:], 6272.0, None, OP.mult)
                nc.sync.dma_start(ar_in[0][:], pk[0][:])
                nc.gpsimd.collective_compute(
                    "AllReduce", OP.add, replica_groups=[list(range(N_CORES))],
                    ins=[ar_in[0].opt()], outs=[ar_out[0].opt()])
                nc.sync.dma_start(gpk[0][:], ar_out[0][:])
                for c in range(2):
                    bn_coeffs(0, c, gpk[0][:, 2 * c:2 * c + 1],
                              gpk[0][:, 2 * c + 1:2 * c + 2], None)

            # ---------- quantize: z = a*in + b (ACT), clip->int8, ->fp8 ----------
            def quantize_block(src_ap, dst_ap, a, b, eng, names):
                z = qtp.tile([P, 14, H], F32, name=names + "z", tag="qz", bufs=3)
                nc.scalar.activation(z[:], src_ap, AF.Identity,
                                     bias=b[:], scale=a[:])
                u = qtp.tile([P, 14, H], I8, name=names + "u", tag="qu", bufs=3)
                eng.tensor_scalar(u[:], z[:], 0.0, 15.0, OP.max, OP.min)
                eng.tensor_copy(dst_ap, u[:])

            with nc.named_scope("quant0"):
                for c in range(2):
                    for img in range(IMG):
                        for blk in range(4):
                            h0 = blk * 14
                            xt = qtp.tile([P, 14, H], F32, name=f"qx{c}{img}{blk}",
                                          tag="qin", bufs=4)
                            nc.sync.dma_start(
                                xt[:], xv[c * P:(c + 1) * P, img, h0:h0 + 14, :])
                            lr = img * 58 + 1 + h0   # logical row
                            dst = act0[c][:, lr:lr + 14, :]
                            eng = nc.vector if (img * 4 + blk) % 2 == 0 else nc.gpsimd
                            quantize_block(xt[:], dst, avec[0][c], bbvec[0][c],
                                           eng, f"q0_{c}{img}{blk}")

            # ---------- conv (shared) ----------
            def conv(v, acts, epilogue):
                for gi, grp in enumerate(GROUPS):
                    for co in range(2):
                        psums = []
                        for wi, (r0, nr) in enumerate(grp):
                            ps = psp.tile([P, nr, H], F32,
                                          name=f"ps{v}_{gi}_{co}_{wi}",
                                          tag="psw", padded_shape=[P, 9, H])
                            psums.append(ps)
                        for ti, (dy, dx) in enumerate(TAPS):
                            tap = dy * 3 + dx
                            wlo, whi = max(0, 1 - dx), min(H, H + 1 - dx)
                            jlo = max(0, dx - 1)
                            for ci in range(2):
                                lhsT = wT[v][ci][:, tap, co * P:(co + 1) * P]
                                first = ti == 0 and ci == 0
                                last = ti == 8 and ci == 1
                                for wi, (r0, nr) in enumerate(grp):
                                    rhs = acts[ci][:, r0 + dy - 1:r0 + dy - 1 + nr,
                                                   jlo:jlo + whi - wlo]
                                    nc.tensor.matmul(
                                        psums[wi][:, :, wlo:whi], lhsT, rhs,
                                        start=first, stop=last)
                        for wi, (r0, nr) in enumerate(grp):
                            epilogue(co, r0, nr, psums[wi])

            # ---------- conv0 + epilogue (spill S + interior sums) ----------
            slot_idx = [0, 0]

            def epi0(co, r0, nr, ps):
                for (rl, n, img, h0) in _runs(r0, nr):
                    sl = ps[:, rl - r0:rl - r0 + n, :]
                    tmp = runp.tile([P, n, H], F32, name=f"s_{co}_{rl}",
                                    tag="srun", bufs=6, padded_shape=[P, 9, H])
                    k = slot_idx[co]
                    slot_idx[co] += 1
                    nc.scalar.activation(tmp[:], sl, AF.Identity,
                                         accum_out=sumS[co][:, k:k + 1])
                    sq = runp.tile([P, n, H], F32, name=f"sq_{co}_{rl}",
                                   tag="sq", bufs=4, padded_shape=[P, 9, H])
                    nc.vector.scalar_tensor_tensor(
                        sq[:], tmp[:], 0.0, tmp[:], OP.bypass, OP.mult,
                        accum_out=sqS[co][:, k:k + 1])
                    nc.sync.dma_start(s_dram[co][:, img, h0:h0 + n, :], tmp[:])

            with nc.named_scope("conv0"):
                conv(0, act0, epi0)

            # ---------- BN1 stats + AR + coeffs ----------
            with nc.named_scope("stats1"):
                for c in range(2):
                    ns = slot_idx[c]
                    nc.vector.tensor_reduce(pk[1][:, 2 * c:2 * c + 1],
                                            sumS[c][:, 0:ns], AX.X, OP.add)
                    nc.vector.tensor_reduce(pk[1][:, 2 * c + 1:2 * c + 2],
                                            sqS[c][:, 0:ns], AX.X, OP.add)
                nc.sync.dma_start(ar_in[1][:], pk[1][:])
                nc.gpsimd.collective_compute(
                    "AllReduce", OP.add, replica_groups=[list(range(N_CORES))],
                    ins=[ar_in[1].opt()], outs=[ar_out[1].opt()])
                nc.sync.dma_start(gpk[1][:], ar_out[1][:])
                for c in range(2):
                    bn_coeffs(1, c, gpk[1][:, 2 * c:2 * c + 1],
                              gpk[1][:, 2 * c + 1:2 * c + 2], svec[0])

            # ---------- quantize1: S -> act1 codes ----------
            with nc.named_scope("quant1"):
                for c in range(2):
                    for img in range(IMG):
                        for blk in range(4):
                            h0 = blk * 14
                            st = qtp.tile([P, 14, H], F32, name=f"qs{c}{img}{blk}",
                                          tag="qin", bufs=4)
                            nc.sync.dma_start(
                                st[:], s_dram[c][:, img, h0:h0 + 14, :])
                            lr = img * 58 + 1 + h0
                            dst = act1[c][:, lr:lr + 14, :]
                            eng = nc.vector if (img * 4 + blk) % 2 == 0 else nc.gpsimd
                            quantize_block(st[:], dst, avec[1][c], bbvec[1][c],
                                           eng, f"q1_{c}{img}{blk}")

            # ---------- conv1 + residual epilogue ----------
            def epi1(co, r0, nr, ps):
                for (rl, n, img, h0) in _runs(r0, nr):
                    sl = ps[:, rl - r0:rl - r0 + n, :]
                    xt = runp.tile([P, n, H], F32, name=f"x_{co}_{rl}",
                                   tag="xrun", bufs=6, padded_shape=[P, 9, H])
                    nc.sync.dma_start(
                        xt[:], xv[co * P:(co + 1) * P, img, h0:h0 + n, :])
                    ot = runp.tile([P, n, H], F32, name=f"o_{co}_{rl}",
                                   tag="srun", bufs=6, padded_shape=[P, 9, H])
                    nc.vector.scalar_tensor_tensor(
                        ot[:], sl, svec[1][:], xt[:], OP.mult, OP.add)
                    nc.sync.dma_start(
                        ov[co * P:(co + 1) * P, img, h0:h0 + n, :], ot[:])

            with nc.named_scope("conv1"):
                conv(1, act1, epi1)

    nc.compile()
    return nc


def _install_ntff_hook():
    """Provide antenv.axon_hooks (absent in this image) via ctypes so that
    run_bass_kernel_spmd(trace=True) can capture NTFF profiles."""
    try:
        from antenv.axon_hooks import get_axon_ntff_profile_hook  # noqa: F401
        return
    except ImportError:
        pass
    import contextlib
    import ctypes
    import types

    so_path = "/opt/axon/libaxon_pjrt.so"
    if not os.path.exists(so_path):
        return
    lib = ctypes.CDLL(so_path)
    if not hasattr(lib, "axon_start_nrt_profile"):
        return
    lib.axon_start_nrt_profile.argtypes = [ctypes.POINTER(ctypes.c_int64),
                                           ctypes.c_size_t]
    lib.axon_start_nrt_profile.restype = ctypes.c_int64
    lib.axon_stop_nrt_profile.argtypes = [ctypes.c_char_p]
    lib.axon_stop_nrt_profile.restype = ctypes.c_int64

    @contextlib.contextmanager
    def _hook(output_dir, device_ids):
        import jax
        jax.devices()
        if device_ids:
            ids = (ctypes.c_int64 * len(device_ids))(*device_ids)
            rc = lib.axon_start_nrt_profile(ids, len(device_ids))
        else:
            rc = lib.axon_start_nrt_profile(None, 0)
        if rc != 0:
            raise RuntimeError(f"axon_start_nrt_profile rc={rc}")
        try:
            yield
        finally:
            n = lib.axon_stop_nrt_profile(str(output_dir).encode())
            print(f"ntff profile: {n} file(s) written to {output_dir}")

    hook_holder = [_hook]
    mod = types.ModuleType("antenv.axon_hooks")
    mod.get_axon_ntff_profile_hook = lambda: hook_holder[0]
    mod.set_axon_ntff_profile_hook = lambda h: hook_holder.__setitem__(0, h)
    import antenv
    sys.modules["antenv.axon_hooks"] = mod
    antenv.axon_hooks = mod


_NC = None


def _get_nc():
    global _NC
    if _NC is None:
        _NC = build()
    return _NC


LAST_RESULTS = None


def kernel(x, bn0_gamma, bn0_beta, conv0_w, bn1_gamma, bn1_beta, conv1_w):
    global LAST_RESULTS
    nc = _get_nc()
    shared = {
        # permute OIHW -> [i, kh, kw, o] so on-chip weight access is contiguous
        "conv0_w": np.ascontiguousarray(
            np.asarray(conv0_w, np.float32).transpose(1, 2, 3, 0)),
        "conv1_w": np.ascontiguousarray(
            np.asarray(conv1_w, np.float32).transpose(1, 2, 3, 0)),
        "bn0_gamma": np.ascontiguousarray(bn0_gamma, np.float32),
        "bn0_beta": np.ascontiguousarray(bn0_beta, np.float32),
        "bn1_gamma": np.ascontiguousarray(bn1_gamma, np.float32),
        "bn1_beta": np.ascontiguousarray(bn1_beta, np.float32),
    }
    x = np.ascontiguousarray(x, np.float32)
    in_maps = [{"x": x[2 * c:2 * c + 2], **shared} for c in range(N_CORES)]
    trace = bool(int(os.environ.get("KERNEL_TRACE", "0")))
    if trace:
        _install_ntff_hook()
    res = bass_utils.run_bass_kernel_spmd(
        nc, in_maps, core_ids=list(range(N_CORES)), trace=trace)
    LAST_RESULTS = res
    return np.concatenate([res.results[c]["out"] for c in range(N_CORES)], axis=0)


# revision 6
# speedup vs baseline: 1.3923x; 1.0344x over previous
"""Trainium2 Bass kernel: PreActBlock with DoReFa 4-bit quantization (sync-BN).

  out = conv3x3(q(relu(BN1(conv3x3(q(relu(BN0(x))), qw(w0))))), qw(w1)) + x

Design (8 cores, data-parallel over batch 16 -> 2 images/core):
 - Quantized activations are integers 0..15 and quantized weights odd integers
   -15..15 (x scale).  Both are exact in fp8e4 (e4m3) and the PE accumulates
   in fp32, so every conv is computed EXACTLY as integer sums (|S| < 2^20).
 - BN batch stats are all-reduced across the 8 cores (sync-BN semantics).
 - Rounding: fp32->int8 cast is round-to-nearest-even (HW-validated), so
   quantize = clip(tensor_scalar max/min -> int8) + int8->fp8 copy.
 - Spatial layout: unpadded 56-wide rows; 2 images stacked with zero pad rows
   (116 rows).  Column zero-padding is implicit: edge taps run one column
   narrower (the skipped outputs receive exactly the zero-pad contribution).
 - Conv: per 9-row window (N = 504 <= 512, one PSUM bank) accumulate
   9 taps x 2 ci-chunks of shifted matmuls; weights host-permuted to
   [i, kh, kw, o] so weight DMA and quantized code writes are contiguous.
"""
import os
import sys

sys.path.insert(0, "/opt/trn_rl_repo")

import numpy as np

import concourse.bacc as bacc
import concourse.bass as bass
import concourse.mybir as mybir
from concourse import tile
from concourse import bass_utils

F32 = mybir.dt.float32
FP8 = mybir.dt.float8e4
I8 = mybir.dt.int8
AX = mybir.AxisListType
OP = mybir.AluOpType
AF = mybir.ActivationFunctionType

P = 128
N_CORES = 8
IMG = 2              # images per core
H = 56
ROWS = 116           # 2 images x (1 pad + 56 + 1 pad) rows
CNT = 50176.0        # global BN count: 16 * 56 * 56
EPS = 1e-5

# 9-row output windows over logical rows 1..114
WINDOWS = [(1 + 9 * k, 9) for k in range(12)] + [(109, 6)]
GROUPS = [WINDOWS[0:4], WINDOWS[4:8], WINDOWS[8:12], WINDOWS[12:13]]
# tap order: full-width tap (dy=0,dx=1) first so start=True covers all columns
TAPS = [(0, 1), (0, 0), (0, 2), (1, 0), (1, 1), (1, 2), (2, 0), (2, 1), (2, 2)]


def _runs(r0, nr):
    """Interior row-runs of a window: (logical_row, nrows, img, h0)."""
    out = []
    for lo, hi, img, base in ((1, 56, 0, 1), (59, 114, 1, 59)):
        a, b = max(r0, lo), min(r0 + nr - 1, hi)
        if a <= b:
            out.append((a, b - a + 1, img, a - base))
    return out


def build():
    nc = bacc.Bacc("TRN2", target_bir_lowering=False, debug=False,
                   enable_asserts=False, num_devices=N_CORES)

    x_d = nc.dram_tensor("x", [IMG, 256, H, H], F32, kind="ExternalInput")
    # host-permuted to [i, kh, kw, o]
    w_d = [nc.dram_tensor("conv0_w", [256, 3, 3, 256], F32, kind="ExternalInput"),
           nc.dram_tensor("conv1_w", [256, 3, 3, 256], F32, kind="ExternalInput")]
    g_d = [nc.dram_tensor("bn0_gamma", [256], F32, kind="ExternalInput"),
           nc.dram_tensor("bn1_gamma", [256], F32, kind="ExternalInput")]
    b_d = [nc.dram_tensor("bn0_beta", [256], F32, kind="ExternalInput"),
           nc.dram_tensor("bn1_beta", [256], F32, kind="ExternalInput")]
    out_d = nc.dram_tensor("out", [IMG, 256, H, H], F32, kind="ExternalOutput")

    xv = x_d.ap().rearrange("n c h w -> c n h w")       # [256, 2, 56, 56]
    ov = out_d.ap().rearrange("n c h w -> c n h w")

    with tile.TileContext(nc) as tc:
        with tc.tile_pool(name="act", bufs=1) as actp, \
             tc.tile_pool(name="wtp", bufs=1) as wtp, \
             tc.tile_pool(name="wq", bufs=4) as wqp, \
             tc.tile_pool(name="qt", bufs=3) as qtp, \
             tc.tile_pool(name="run", bufs=6) as runp, \
             tc.tile_pool(name="st", bufs=1) as stp, \
             tc.tile_pool(name="ps", bufs=8, space="PSUM") as psp, \
             tc.tile_pool(name="dram", bufs=1, space="DRAM") as drp:

            # ---------- static tiles ----------
            act0 = [actp.tile([P, ROWS, H], FP8, name=f"act0_{c}") for c in range(2)]
            act1 = [actp.tile([P, ROWS, H], FP8, name=f"act1_{c}") for c in range(2)]
            # quantized weight codes, [ci, tap, co] fp8
            wT = [[wtp.tile([P, 9, 256], FP8, name=f"w{v}T_{c}") for c in range(2)]
                  for v in range(2)]
            # DRAM spill of conv0 integer outputs (unpadded interior)
            s_dram = [drp.tile([P, IMG, H, H], F32, name=f"s_dram_{c}")
                      for c in range(2)]
            ar_in = [drp.tile([P, 4], F32, name=f"ar_in_{i}") for i in range(2)]
            ar_out = [drp.tile([P, 4], F32, name=f"ar_out_{i}") for i in range(2)]

            # stats / small vectors
            xbn = [stp.tile([P, 16, 6], F32, name=f"xbn_{c}") for c in range(2)]
            sumS = [stp.tile([P, 16], F32, name=f"sumS_{c}") for c in range(2)]
            sqS = [stp.tile([P, 16], F32, name=f"sqS_{c}") for c in range(2)]
            gvec = [[stp.tile([P, 1], F32, name=f"g{v}_{c}") for c in range(2)]
                    for v in range(2)]
            bvec = [[stp.tile([P, 1], F32, name=f"b{v}_{c}") for c in range(2)]
                    for v in range(2)]
            avec = [[stp.tile([P, 1], F32, name=f"a{v}_{c}") for c in range(2)]
                    for v in range(2)]
            bbvec = [[stp.tile([P, 1], F32, name=f"bb{v}_{c}") for c in range(2)]
                     for v in range(2)]
            svec = [stp.tile([P, 1], F32, name=f"scale_{v}") for v in range(2)]
            pk = [stp.tile([P, 4], F32, name=f"pk_{i}") for i in range(2)]
            gpk = [stp.tile([P, 4], F32, name=f"gpk_{i}") for i in range(2)]

            def vtile(name):
                return stp.tile([P, 1], F32, name=name, tag="vtmp", bufs=8)

            # ---------- tiny vector helpers (all on [P,1]) ----------
            def refined_rsqrt(vpe, name):
                """rsqrt(vpe) with 2 Newton refinements (vpe > 0)."""
                r = vtile(f"{name}_r")
                nc.vector.reciprocal(r[:], vpe[:])
                y = vtile(f"{name}_y")
                nc.scalar.activation(y[:], r[:], AF.Sqrt)
                for i in range(2):
                    y2 = vtile(f"{name}_y2{i}")
                    nc.vector.tensor_mul(y2[:], y[:], y[:])
                    t2 = vtile(f"{name}_t2{i}")
                    nc.vector.tensor_mul(t2[:], vpe[:], y2[:])
                    h = vtile(f"{name}_h{i}")
                    nc.vector.tensor_scalar(h[:], t2[:], -0.5, 1.5, OP.mult, OP.add)
                    yn = vtile(f"{name}_yn{i}")
                    nc.vector.tensor_mul(yn[:], y[:], h[:])
                    y = yn
                return y

            def bn_coeffs(v, c, gsum, gsumsq, scale):
                """a,b for z = a*S + b  (= 15 * BN-affine), scale=None for BN0."""
                mean = vtile(f"m{v}{c}")
                nc.vector.tensor_scalar(mean[:], gsum, 1.0 / CNT, None, OP.mult)
                ex2 = vtile(f"e{v}{c}")
                nc.vector.tensor_scalar(ex2[:], gsumsq, 1.0 / CNT, None, OP.mult)
                m2 = vtile(f"m2{v}{c}")
                nc.vector.tensor_mul(m2[:], mean[:], mean[:])
                var = vtile(f"va{v}{c}")
                nc.vector.tensor_sub(var[:], ex2[:], m2[:])
                if scale is not None:
                    s2 = vtile(f"s2{v}{c}")
                    nc.vector.tensor_mul(s2[:], scale[:], scale[:])
                    nc.vector.tensor_mul(var[:], var[:], s2[:])
                    mo = vtile(f"mo{v}{c}")
                    nc.vector.tensor_mul(mo[:], mean[:], scale[:])
                    mean = mo
                vpe = vtile(f"vp{v}{c}")
                nc.vector.tensor_scalar(vpe[:], var[:], EPS, None, OP.add)
                rs = refined_rsqrt(vpe, f"rs{v}{c}")
                grs = vtile(f"gr{v}{c}")
                nc.vector.tensor_mul(grs[:], gvec[v][c][:], rs[:])
                a = avec[v][c]
                if scale is not None:
                    asc = vtile(f"as{v}{c}")
                    nc.vector.tensor_mul(asc[:], grs[:], scale[:])
                    nc.vector.tensor_scalar(a[:], asc[:], 15.0, None, OP.mult)
                else:
                    nc.vector.tensor_scalar(a[:], grs[:], 15.0, None, OP.mult)
                mg = vtile(f"mg{v}{c}")
                nc.vector.tensor_mul(mg[:], mean[:], grs[:])
                mg15 = vtile(f"mh{v}{c}")
                nc.vector.tensor_scalar(mg15[:], mg[:], 15.0, None, OP.mult)
                b15 = vtile(f"bh{v}{c}")
                nc.vector.tensor_scalar(b15[:], bvec[v][c][:], 15.0, None, OP.mult)
                nc.vector.tensor_sub(bbvec[v][c][:], b15[:], mg15[:])

            # ---------- load BN params ----------
            for v in range(2):
                for c in range(2):
                    nc.sync.dma_start(gvec[v][c][:], g_d[v].ap()[c * P:(c + 1) * P])
                    nc.sync.dma_start(bvec[v][c][:], b_d[v].ap()[c * P:(c + 1) * P])

            # ---------- act pad-row zeroing ----------
            with nc.named_scope("memset"):
                for t in act0 + act1:
                    for r in (0, 57, 58, 115):
                        nc.gpsimd.memset(t[:, r, :], 0.0)

            # ---------- weight quantization (both convs) ----------
            # DRAM layout [i, kh, kw, o] -> contiguous [ci, tap, co] tiles
            with nc.named_scope("wquant"):
                mxp = [stp.tile([P, 4], F32, name=f"mxp_{v}") for v in range(2)]
                wnat = {}
                for v in range(2):
                    wv = w_d[v].ap().rearrange("i kh kw o -> i (kh kw) o")
                    for c in range(2):
                        for hh in range(2):  # tap halves: 0 -> taps 0..3, 1 -> 4..8
                            t0, t1 = (0, 4) if hh == 0 else (4, 9)
                            wn = wqp.tile([P, t1 - t0, 256], F32,
                                          name=f"wn{v}{c}{hh}", tag="wnat", bufs=4,
                                          padded_shape=[P, 5, 256])
                            nc.sync.dma_start(
                                wn[:], wv[c * P:(c + 1) * P, t0:t1, :])
                            wnat[(v, c, hh)] = wn
                for v in range(2):
                    for i, (c, hh) in enumerate(((0, 0), (0, 1), (1, 0), (1, 1))):
                        wn = wnat[(v, c, hh)]
                        t = wqp.tile(list(wn.shape), F32, name=f"t{v}{c}{hh}",
                                     tag=f"tanh{v}", bufs=4,
                                     padded_shape=[P, 5, 256])
                        tf = t.rearrange("p a b -> p (a b)")
                        wf = wn.rearrange("p a b -> p (a b)")
                        nc.scalar.activation(tf[:], wf[:], AF.Tanh)
                        nc.vector.tensor_reduce(
                            mxp[v][:, i:i + 1], tf[:], AX.X, OP.max,
                            apply_absolute_value=True)
                        wnat[(v, c, hh, "t")] = t
                    mx1 = vtile(f"mx1_{v}")
                    nc.vector.tensor_reduce(mx1[:], mxp[v][:], AX.X, OP.max,
                                            apply_absolute_value=True)
                    msc = stp.tile([1, 1], F32, name=f"msc_{v}")
                    nc.gpsimd.tensor_reduce(msc[:], mx1[:], AX.C, OP.max)
                    mvec = vtile(f"mvec_{v}")
                    nc.gpsimd.partition_broadcast(mvec[:], msc[:])
                    # svec = M/225 (psum scale); r = 7.5/M for codes
                    nc.vector.tensor_scalar(svec[v][:], mvec[:], 1.0 / 225.0,
                                            None, OP.mult)
                    r = vtile(f"rin_{v}")
                    nc.vector.reciprocal(r[:], mvec[:])
                    for i in range(2):  # Newton: r = r*(2 - M*r)
                        t1 = vtile(f"rn1_{v}{i}")
                        nc.vector.tensor_mul(t1[:], mvec[:], r[:])
                        t2 = vtile(f"rn2_{v}{i}")
                        nc.vector.tensor_scalar(t2[:], t1[:], -1.0, 2.0,
                                                OP.mult, OP.add)
                        rn = vtile(f"rn3_{v}{i}")
                        nc.vector.tensor_mul(rn[:], r[:], t2[:])
                        r = rn
                    sc = vtile(f"sc_{v}")
                    nc.vector.tensor_scalar(sc[:], r[:], 7.5, None, OP.mult)
                    for i, (c, hh) in enumerate(((0, 0), (0, 1), (1, 0), (1, 1))):
                        eng = nc.vector if i % 2 == 0 else nc.gpsimd
                        t = wnat[(v, c, hh, "t")]
                        sh = list(t.shape)
                        z = wqp.tile(sh, F32, name=f"z{v}{c}{hh}", tag="wz",
                                     bufs=4, padded_shape=[P, 5, 256])
                        tf = t.rearrange("p a b -> p (a b)")
                        zf = z.rearrange("p a b -> p (a b)")
                        eng.tensor_scalar(zf[:], tf[:], sc[:], 7.5, OP.mult, OP.add)
                        ri = wqp.tile(sh, I8, name=f"ri{v}{c}{hh}", tag="wr",
                                      bufs=4, padded_shape=[P, 5, 256])
                        rf = ri.rearrange("p a b -> p (a b)")
                        eng.tensor_scalar(rf[:], zf[:], 0.0, 15.0, OP.max, OP.min)
                        t0 = 0 if hh == 0 else 4
                        wTf = wT[v][c].rearrange("p a b -> p (a b)")
                        dst = wTf[:, t0 * 256:(t0 + sh[1]) * 256]
                        eng.tensor_scalar(dst, rf[:], 2.0, -15.0, OP.mult, OP.add)

            # ---------- BN0 stats over x (streamed) ----------
            with nc.named_scope("stats0"):
                for c in range(2):
                    for img in range(IMG):
                        for blk in range(4):
                            h0 = blk * 14
                            xt = qtp.tile([P, 14, H], F32, name=f"sx{c}{img}{blk}",
                                          tag="qin", bufs=4)
                            nc.sync.dma_start(
                                xt[:], xv[c * P:(c + 1) * P, img, h0:h0 + 14, :])
                            fl = xt.rearrange("p a b -> p (a b)")
                            k = (img * 4 + blk) * 2
                            nc.vector.bn_stats(xbn[c][:, k, :], fl[:, 0:392])
                            nc.vector.bn_stats(xbn[c][:, k + 1, :], fl[:, 392:784])
                    mv = stp.tile([P, 2], F32, name=f"mv0_{c}")
                    nc.vector.bn_aggr(mv[:], xbn[c][:, 0:16, :])
                    # local sums: n*(mean, var+mean^2), n = 6272
                    nc.vector.tensor_scalar(pk[0][:, 2 * c:2 * c + 1],
                                            mv[:, 0:1], 6272.0, None, OP.mult)
                    m2 = vtile(f"xm2_{c}")
                    nc.vector.tensor_mul(m2[:], mv[:, 0:1], mv[:, 0:1])
                    vp = vtile(f"xvp_{c}")
                    nc.vector.tensor_add(vp[:], mv[:, 1:2], m2[:])
                    nc.vector.tensor_scalar(pk[0][:, 2 * c + 1:2 * c + 2],
                                            vp[:], 6272.0, None, OP.mult)
                nc.sync.dma_start(ar_in[0][:], pk[0][:])
                nc.gpsimd.collective_compute(
                    "AllReduce", OP.add, replica_groups=[list(range(N_CORES))],
                    ins=[ar_in[0].opt()], outs=[ar_out[0].opt()])
                nc.sync.dma_start(gpk[0][:], ar_out[0][:])
                for c in range(2):
                    bn_coeffs(0, c, gpk[0][:, 2 * c:2 * c + 1],
                              gpk[0][:, 2 * c + 1:2 * c + 2], None)

            # ---------- quantize: z = a*in + b (ACT), clip->int8, ->fp8 ----------
            def quantize_block(src_ap, dst_ap, a, b, eng, names):
                z = qtp.tile([P, 14 * H], F32, name=names + "z", tag="qz", bufs=3)
                nc.scalar.activation(z[:], src_ap, AF.Identity,
                                     bias=b[:], scale=a[:])
                u = qtp.tile([P, 14 * H], I8, name=names + "u", tag="qu", bufs=3)
                eng.tensor_scalar(u[:], z[:], 0.0, 15.0, OP.max, OP.min)
                eng.tensor_copy(dst_ap, u[:])

            with nc.named_scope("quant0"):
                for c in range(2):
                    for img in range(IMG):
                        for blk in range(4):
                            h0 = blk * 14
                            xt = qtp.tile([P, 14, H], F32, name=f"qx{c}{img}{blk}",
                                          tag="qin", bufs=4)
                            nc.sync.dma_start(
                                xt[:], xv[c * P:(c + 1) * P, img, h0:h0 + 14, :])
                            lr = img * 58 + 1 + h0   # logical row
                            a0f = act0[c].rearrange("p r c -> p (r c)")
                            dst = a0f[:, lr * H:(lr + 14) * H]
                            eng = nc.vector if (img * 4 + blk) % 2 == 0 else nc.gpsimd
                            quantize_block(xt.rearrange("p a b -> p (a b)")[:],
                                           dst, avec[0][c], bbvec[0][c],
                                           eng, f"q0_{c}{img}{blk}")

            # ---------- conv (shared) ----------
            def conv(v, acts, epilogue):
                aflat = [acts[c].rearrange("p r c -> p (r c)") for c in range(2)]
                for gi, grp in enumerate(GROUPS):
                    for co in range(2):
                        psums = []
                        for wi, (r0, nr) in enumerate(grp):
                            ps = psp.tile([P, nr, H], F32,
                                          name=f"ps{v}_{gi}_{co}_{wi}",
                                          tag="psw", padded_shape=[P, 9, H])
                            psums.append(ps)
                        for ti, (dy, dx) in enumerate(TAPS):
                            tap = dy * 3 + dx
                            wlo, whi = max(0, 1 - dx), min(H, H + 1 - dx)
                            jlo = max(0, dx - 1)
                            for ci in range(2):
                                lhsT = wT[v][ci][:, tap, co * P:(co + 1) * P]
                                first = ti == 0 and ci == 0
                                last = ti == 8 and ci == 1
                                for wi, (r0, nr) in enumerate(grp):
                                    if dx == 1:
                                        rr = (r0 + dy - 1) * H
                                        rhs = aflat[ci][:, rr:rr + nr * H]
                                        out = psums[wi].rearrange(
                                            "p r c -> p (r c)")[:, 0:nr * H]
                                    else:
                                        rhs = acts[ci][:, r0 + dy - 1:
                                                       r0 + dy - 1 + nr,
                                                       jlo:jlo + whi - wlo]
                                        out = psums[wi][:, :, wlo:whi]
                                    nc.tensor.matmul(out, lhsT, rhs,
                                                     start=first, stop=last)
                        for wi, (r0, nr) in enumerate(grp):
                            epilogue(co, r0, nr, psums[wi])

            # ---------- conv0 + epilogue (spill S + interior sums) ----------
            slot_idx = [0, 0]

            def epi0(co, r0, nr, ps):
                psf = ps.rearrange("p r c -> p (r c)")
                for (rl, n, img, h0) in _runs(r0, nr):
                    sl = psf[:, (rl - r0) * H:(rl - r0 + n) * H]
                    tmp = runp.tile([P, n * H], F32, name=f"s_{co}_{rl}",
                                    tag="srun", bufs=6, padded_shape=[P, 9 * H])
                    k = slot_idx[co]
                    slot_idx[co] += 1
                    nc.scalar.activation(tmp[:], sl, AF.Identity,
                                         accum_out=sumS[co][:, k:k + 1])
                    sq = runp.tile([P, n * H], F32, name=f"sq_{co}_{rl}",
                                   tag="sq", bufs=4, padded_shape=[P, 9 * H])
                    nc.vector.scalar_tensor_tensor(
                        sq[:], tmp[:], 0.0, tmp[:], OP.bypass, OP.mult,
                        accum_out=sqS[co][:, k:k + 1])
                    nc.sync.dma_start(s_dram[co][:, img, h0:h0 + n, :], tmp[:])

            with nc.named_scope("conv0"):
                conv(0, act0, epi0)

            # ---------- BN1 stats + AR + coeffs ----------
            with nc.named_scope("stats1"):
                for c in range(2):
                    ns = slot_idx[c]
                    nc.vector.tensor_reduce(pk[1][:, 2 * c:2 * c + 1],
                                            sumS[c][:, 0:ns], AX.X, OP.add)
                    nc.vector.tensor_reduce(pk[1][:, 2 * c + 1:2 * c + 2],
                                            sqS[c][:, 0:ns], AX.X, OP.add)
                nc.sync.dma_start(ar_in[1][:], pk[1][:])
                nc.gpsimd.collective_compute(
                    "AllReduce", OP.add, replica_groups=[list(range(N_CORES))],
                    ins=[ar_in[1].opt()], outs=[ar_out[1].opt()])
                nc.sync.dma_start(gpk[1][:], ar_out[1][:])
                for c in range(2):
                    bn_coeffs(1, c, gpk[1][:, 2 * c:2 * c + 1],
                              gpk[1][:, 2 * c + 1:2 * c + 2], svec[0])

            # ---------- quantize1: S -> act1 codes ----------
            with nc.named_scope("quant1"):
                for c in range(2):
                    for img in range(IMG):
                        for blk in range(4):
                            h0 = blk * 14
                            st = qtp.tile([P, 14, H], F32, name=f"qs{c}{img}{blk}",
                                          tag="qin", bufs=4)
                            nc.sync.dma_start(
                                st[:], s_dram[c][:, img, h0:h0 + 14, :])
                            lr = img * 58 + 1 + h0
                            a1f = act1[c].rearrange("p r c -> p (r c)")
                            dst = a1f[:, lr * H:(lr + 14) * H]
                            eng = nc.vector if (img * 4 + blk) % 2 == 0 else nc.gpsimd
                            quantize_block(st.rearrange("p a b -> p (a b)")[:],
                                           dst, avec[1][c], bbvec[1][c],
                                           eng, f"q1_{c}{img}{blk}")

            # ---------- conv1 + residual epilogue ----------
            def epi1(co, r0, nr, ps):
                psf = ps.rearrange("p r c -> p (r c)")
                for (rl, n, img, h0) in _runs(r0, nr):
                    sl = psf[:, (rl - r0) * H:(rl - r0 + n) * H]
                    xt = runp.tile([P, n * H], F32, name=f"x_{co}_{rl}",
                                   tag="xrun", bufs=6, padded_shape=[P, 9 * H])
                    nc.sync.dma_start(
                        xt[:], xv[co * P:(co + 1) * P, img, h0:h0 + n, :])
                    ot = runp.tile([P, n * H], F32, name=f"o_{co}_{rl}",
                                   tag="srun", bufs=6, padded_shape=[P, 9 * H])
                    nc.vector.scalar_tensor_tensor(
                        ot[:], sl, svec[1][:], xt[:], OP.mult, OP.add)
                    nc.sync.dma_start(
                        ov[co * P:(co + 1) * P, img, h0:h0 + n, :],
                        ot.rearrange("p (a b) -> p a b", b=H)[:])

            with nc.named_scope("conv1"):
                conv(1, act1, epi1)

    nc.compile()
    return nc


def _install_ntff_hook():
    """Provide antenv.axon_hooks (absent in this image) via ctypes so that
    run_bass_kernel_spmd(trace=True) can capture NTFF profiles."""
    try:
        from antenv.axon_hooks import get_axon_ntff_profile_hook  # noqa: F401
        return
    except ImportError:
        pass
    import contextlib
    import ctypes
    import types

    so_path = "/opt/axon/libaxon_pjrt.so"
    if not os.path.exists(so_path):
        return
    lib = ctypes.CDLL(so_path)
    if not hasattr(lib, "axon_start_nrt_profile"):
        return
    lib.axon_start_nrt_profile.argtypes = [ctypes.POINTER(ctypes.c_int64),
                                           ctypes.c_size_t]
    lib.axon_start_nrt_profile.restype = ctypes.c_int64
    lib.axon_stop_nrt_profile.argtypes = [ctypes.c_char_p]
    lib.axon_stop_nrt_profile.restype = ctypes.c_int64

    @contextlib.contextmanager
    def _hook(output_dir, device_ids):
        import jax
        jax.devices()
        if device_ids:
            ids = (ctypes.c_int64 * len(device_ids))(*device_ids)
            rc = lib.axon_start_nrt_profile(ids, len(device_ids))
        else:
            rc = lib.axon_start_nrt_profile(None, 0)
        if rc != 0:
            raise RuntimeError(f"axon_start_nrt_profile rc={rc}")
        try:
            yield
        finally:
            n = lib.axon_stop_nrt_profile(str(output_dir).encode())
            print(f"ntff profile: {n} file(s) written to {output_dir}")

    hook_holder = [_hook]
    mod = types.ModuleType("antenv.axon_hooks")
    mod.get_axon_ntff_profile_hook = lambda: hook_holder[0]
    mod.set_axon_ntff_profile_hook = lambda h: hook_holder.__setitem__(0, h)
    import antenv
    sys.modules["antenv.axon_hooks"] = mod
    antenv.axon_hooks = mod


_NC = None


def _get_nc():
    global _NC
    if _NC is None:
        _NC = build()
    return _NC


LAST_RESULTS = None


def kernel(x, bn0_gamma, bn0_beta, conv0_w, bn1_gamma, bn1_beta, conv1_w):
    global LAST_RESULTS
    nc = _get_nc()
    shared = {
        # permute OIHW -> [i, kh, kw, o] so on-chip weight access is contiguous
        "conv0_w": np.ascontiguousarray(
            np.asarray(conv0_w, np.float32).transpose(1, 2, 3, 0)),
        "conv1_w": np.ascontiguousarray(
            np.asarray(conv1_w, np.float32).transpose(1, 2, 3, 0)),
        "bn0_gamma": np.ascontiguousarray(bn0_gamma, np.float32),
        "bn0_beta": np.ascontiguousarray(bn0_beta, np.float32),
        "bn1_gamma": np.ascontiguousarray(bn1_gamma, np.float32),
        "bn1_beta": np.ascontiguousarray(bn1_beta, np.float32),
    }
    x = np.ascontiguousarray(x, np.float32)
    in_maps = [{"x": x[2 * c:2 * c + 2], **shared} for c in range(N_CORES)]
    trace = bool(int(os.environ.get("KERNEL_TRACE", "0")))
    if trace:
        _install_ntff_hook()
    res = bass_utils.run_bass_kernel_spmd(
        nc, in_maps, core_ids=list(range(N_CORES)), trace=trace)
    LAST_RESULTS = res
    return np.concatenate([res.results[c]["out"] for c in range(N_CORES)], axis=0)


# revision 7
# speedup vs baseline: 2.0200x; 1.4509x over previous
"""Trainium2 Bass kernel: PreActBlock with DoReFa 4-bit quantization (sync-BN).

  out = conv3x3(q(relu(BN1(conv3x3(q(relu(BN0(x))), qw(w0))))), qw(w1)) + x

Design (8 cores, data-parallel over batch 16 -> 2 images/core):
 - Quantized activations are integers 0..15 and quantized weights odd integers
   -15..15 (x scale).  Both are exact in fp8e4 (e4m3) and the PE accumulates
   in fp32, so every conv is computed EXACTLY as integer sums (|S| < 2^20).
 - BN batch stats are all-reduced across the 8 cores (sync-BN semantics).
 - Rounding: fp32->int8 cast is round-to-nearest-even (HW-validated), so
   quantize = clip(tensor_scalar max/min -> int8) + int8->fp8 copy.
 - Spatial layout: unpadded 56-wide rows; 2 images stacked with zero pad rows
   (116 rows).  Column zero-padding is implicit: edge taps run one column
   narrower (the skipped outputs receive exactly the zero-pad contribution).
 - Conv: per 9-row window (N = 504 <= 512, one PSUM bank) accumulate
   9 taps x 2 ci-chunks of shifted matmuls; weights host-permuted to
   [i, kh, kw, o] so weight DMA and quantized code writes are contiguous.
"""
import os
import sys

sys.path.insert(0, "/opt/trn_rl_repo")

import numpy as np

import concourse.bacc as bacc
import concourse.bass as bass
import concourse.mybir as mybir
from concourse import tile
from concourse import bass_utils

F32 = mybir.dt.float32
FP8 = mybir.dt.float8e4
I8 = mybir.dt.int8
AX = mybir.AxisListType
OP = mybir.AluOpType
AF = mybir.ActivationFunctionType

P = 128
N_CORES = 8
IMG = 2              # images per core
H = 56
ROWS = 116           # 2 images x (1 pad + 56 + 1 pad) rows
CNT = 50176.0        # global BN count: 16 * 56 * 56
EPS = 1e-5

# 9-row output windows over logical rows 1..114
WINDOWS = [(1 + 9 * k, 9) for k in range(12)] + [(109, 6)]
GROUPS = [WINDOWS[0:4], WINDOWS[4:8], WINDOWS[8:12], WINDOWS[12:13]]
# tap order: full-width tap (dy=0,dx=1) first so start=True covers all columns
TAPS = [(0, 1), (0, 0), (0, 2), (1, 0), (1, 1), (1, 2), (2, 0), (2, 1), (2, 2)]


def _runs(r0, nr):
    """Interior row-runs of a window: (logical_row, nrows, img, h0)."""
    out = []
    for lo, hi, img, base in ((1, 56, 0, 1), (59, 114, 1, 59)):
        a, b = max(r0, lo), min(r0 + nr - 1, hi)
        if a <= b:
            out.append((a, b - a + 1, img, a - base))
    return out


def build():
    nc = bacc.Bacc("TRN2", target_bir_lowering=False, debug=False,
                   enable_asserts=False, num_devices=N_CORES)

    x_d = nc.dram_tensor("x", [IMG, 256, H, H], F32, kind="ExternalInput")
    # host-permuted to [i, kh, kw, o]
    w_d = [nc.dram_tensor("conv0_w", [256, 3, 3, 256], F32, kind="ExternalInput"),
           nc.dram_tensor("conv1_w", [256, 3, 3, 256], F32, kind="ExternalInput")]
    g_d = [nc.dram_tensor("bn0_gamma", [256], F32, kind="ExternalInput"),
           nc.dram_tensor("bn1_gamma", [256], F32, kind="ExternalInput")]
    b_d = [nc.dram_tensor("bn0_beta", [256], F32, kind="ExternalInput"),
           nc.dram_tensor("bn1_beta", [256], F32, kind="ExternalInput")]
    out_d = nc.dram_tensor("out", [IMG, 256, H, H], F32, kind="ExternalOutput")

    xv = x_d.ap().rearrange("n c h w -> c n h w")       # [256, 2, 56, 56]
    ov = out_d.ap().rearrange("n c h w -> c n h w")

    with tile.TileContext(nc) as tc:
        with tc.tile_pool(name="act", bufs=1) as actp, \
             tc.tile_pool(name="wtp", bufs=1) as wtp, \
             tc.tile_pool(name="wq", bufs=4) as wqp, \
             tc.tile_pool(name="qt", bufs=3) as qtp, \
             tc.tile_pool(name="run", bufs=6) as runp, \
             tc.tile_pool(name="st", bufs=1) as stp, \
             tc.tile_pool(name="ps", bufs=8, space="PSUM") as psp, \
             tc.tile_pool(name="dram", bufs=1, space="DRAM") as drp:

            # ---------- static tiles ----------
            act0 = [actp.tile([P, ROWS, H], FP8, name=f"act0_{c}") for c in range(2)]
            act1 = [actp.tile([P, ROWS, H], FP8, name=f"act1_{c}") for c in range(2)]
            # quantized weight codes, [ci, tap, co] fp8
            wT = [[wtp.tile([P, 9, 256], FP8, name=f"w{v}T_{c}") for c in range(2)]
                  for v in range(2)]
            # DRAM spill of conv0 integer outputs (unpadded interior)
            s_dram = [drp.tile([P, IMG, H, H], F32, name=f"s_dram_{c}")
                      for c in range(2)]
            ar_in = [drp.tile([P, 4], F32, name=f"ar_in_{i}") for i in range(2)]
            ar_out = [drp.tile([P, 4], F32, name=f"ar_out_{i}") for i in range(2)]

            # stats / small vectors
            xbn = [stp.tile([P, 16, 6], F32, name=f"xbn_{c}") for c in range(2)]
            sumS = [stp.tile([P, 16], F32, name=f"sumS_{c}") for c in range(2)]
            sqS = [stp.tile([P, 16], F32, name=f"sqS_{c}") for c in range(2)]
            gvec = [[stp.tile([P, 1], F32, name=f"g{v}_{c}") for c in range(2)]
                    for v in range(2)]
            bvec = [[stp.tile([P, 1], F32, name=f"b{v}_{c}") for c in range(2)]
                    for v in range(2)]
            avec = [[stp.tile([P, 1], F32, name=f"a{v}_{c}") for c in range(2)]
                    for v in range(2)]
            bbvec = [[stp.tile([P, 1], F32, name=f"bb{v}_{c}") for c in range(2)]
                     for v in range(2)]
            svec = [stp.tile([P, 1], F32, name=f"scale_{v}") for v in range(2)]
            pk = [stp.tile([P, 4], F32, name=f"pk_{i}") for i in range(2)]
            gpk = [stp.tile([P, 4], F32, name=f"gpk_{i}") for i in range(2)]

            def vtile(name):
                return stp.tile([P, 1], F32, name=name, tag="vtmp", bufs=8)

            # ---------- tiny vector helpers (all on [P,1]) ----------
            def refined_rsqrt(vpe, name):
                """rsqrt(vpe) with 2 Newton refinements (vpe > 0)."""
                r = vtile(f"{name}_r")
                nc.vector.reciprocal(r[:], vpe[:])
                y = vtile(f"{name}_y")
                nc.scalar.activation(y[:], r[:], AF.Sqrt)
                for i in range(2):
                    y2 = vtile(f"{name}_y2{i}")
                    nc.vector.tensor_mul(y2[:], y[:], y[:])
                    t2 = vtile(f"{name}_t2{i}")
                    nc.vector.tensor_mul(t2[:], vpe[:], y2[:])
                    h = vtile(f"{name}_h{i}")
                    nc.vector.tensor_scalar(h[:], t2[:], -0.5, 1.5, OP.mult, OP.add)
                    yn = vtile(f"{name}_yn{i}")
                    nc.vector.tensor_mul(yn[:], y[:], h[:])
                    y = yn
                return y

            def bn_coeffs(v, c, gsum, gsumsq, scale):
                """a,b for z = a*S + b  (= 15 * BN-affine), scale=None for BN0."""
                mean = vtile(f"m{v}{c}")
                nc.vector.tensor_scalar(mean[:], gsum, 1.0 / CNT, None, OP.mult)
                ex2 = vtile(f"e{v}{c}")
                nc.vector.tensor_scalar(ex2[:], gsumsq, 1.0 / CNT, None, OP.mult)
                m2 = vtile(f"m2{v}{c}")
                nc.vector.tensor_mul(m2[:], mean[:], mean[:])
                var = vtile(f"va{v}{c}")
                nc.vector.tensor_sub(var[:], ex2[:], m2[:])
                if scale is not None:
                    s2 = vtile(f"s2{v}{c}")
                    nc.vector.tensor_mul(s2[:], scale[:], scale[:])
                    nc.vector.tensor_mul(var[:], var[:], s2[:])
                    mo = vtile(f"mo{v}{c}")
                    nc.vector.tensor_mul(mo[:], mean[:], scale[:])
                    mean = mo
                vpe = vtile(f"vp{v}{c}")
                nc.vector.tensor_scalar(vpe[:], var[:], EPS, None, OP.add)
                rs = refined_rsqrt(vpe, f"rs{v}{c}")
                grs = vtile(f"gr{v}{c}")
                nc.vector.tensor_mul(grs[:], gvec[v][c][:], rs[:])
                a = avec[v][c]
                if scale is not None:
                    asc = vtile(f"as{v}{c}")
                    nc.vector.tensor_mul(asc[:], grs[:], scale[:])
                    nc.vector.tensor_scalar(a[:], asc[:], 15.0, None, OP.mult)
                else:
                    nc.vector.tensor_scalar(a[:], grs[:], 15.0, None, OP.mult)
                mg = vtile(f"mg{v}{c}")
                nc.vector.tensor_mul(mg[:], mean[:], grs[:])
                mg15 = vtile(f"mh{v}{c}")
                nc.vector.tensor_scalar(mg15[:], mg[:], 15.0, None, OP.mult)
                b15 = vtile(f"bh{v}{c}")
                nc.vector.tensor_scalar(b15[:], bvec[v][c][:], 15.0, None, OP.mult)
                nc.vector.tensor_sub(bbvec[v][c][:], b15[:], mg15[:])

            # ---------- load BN params ----------
            for v in range(2):
                for c in range(2):
                    nc.sync.dma_start(gvec[v][c][:], g_d[v].ap()[c * P:(c + 1) * P])
                    nc.sync.dma_start(bvec[v][c][:], b_d[v].ap()[c * P:(c + 1) * P])

            # ---------- act pad-row zeroing ----------
            with nc.named_scope("memset"):
                for t in act0 + act1:
                    for r in (0, 57, 58, 115):
                        nc.gpsimd.memset(t[:, r, :], 0.0)

            # ---------- weight quantization (both convs) ----------
            # DRAM layout [i, kh, kw, o] -> contiguous [ci, tap, co] tiles
            with nc.named_scope("wquant"):
                mxp = [stp.tile([P, 4], F32, name=f"mxp_{v}") for v in range(2)]
                wnat = {}
                for v in range(2):
                    wv = w_d[v].ap().rearrange("i kh kw o -> i (kh kw) o")
                    for c in range(2):
                        for hh in range(2):  # tap halves: 0 -> taps 0..3, 1 -> 4..8
                            t0, t1 = (0, 4) if hh == 0 else (4, 9)
                            wn = wqp.tile([P, t1 - t0, 256], F32,
                                          name=f"wn{v}{c}{hh}", tag="wnat", bufs=4,
                                          padded_shape=[P, 5, 256])
                            nc.sync.dma_start(
                                wn[:], wv[c * P:(c + 1) * P, t0:t1, :])
                            wnat[(v, c, hh)] = wn
                for v in range(2):
                    for i, (c, hh) in enumerate(((0, 0), (0, 1), (1, 0), (1, 1))):
                        wn = wnat[(v, c, hh)]
                        t = wqp.tile(list(wn.shape), F32, name=f"t{v}{c}{hh}",
                                     tag=f"tanh{v}", bufs=4,
                                     padded_shape=[P, 5, 256])
                        tf = t.rearrange("p a b -> p (a b)")
                        wf = wn.rearrange("p a b -> p (a b)")
                        nc.scalar.activation(tf[:], wf[:], AF.Tanh)
                        nc.vector.tensor_reduce(
                            mxp[v][:, i:i + 1], tf[:], AX.X, OP.max,
                            apply_absolute_value=True)
                        wnat[(v, c, hh, "t")] = t
                    mx1 = vtile(f"mx1_{v}")
                    nc.vector.tensor_reduce(mx1[:], mxp[v][:], AX.X, OP.max,
                                            apply_absolute_value=True)
                    msc = stp.tile([1, 1], F32, name=f"msc_{v}")
                    nc.gpsimd.tensor_reduce(msc[:], mx1[:], AX.C, OP.max)
                    mvec = vtile(f"mvec_{v}")
                    nc.gpsimd.partition_broadcast(mvec[:], msc[:])
                    # svec = M/225 (psum scale); r = 7.5/M for codes
                    nc.vector.tensor_scalar(svec[v][:], mvec[:], 1.0 / 225.0,
                                            None, OP.mult)
                    r = vtile(f"rin_{v}")
                    nc.vector.reciprocal(r[:], mvec[:])
                    for i in range(2):  # Newton: r = r*(2 - M*r)
                        t1 = vtile(f"rn1_{v}{i}")
                        nc.vector.tensor_mul(t1[:], mvec[:], r[:])
                        t2 = vtile(f"rn2_{v}{i}")
                        nc.vector.tensor_scalar(t2[:], t1[:], -1.0, 2.0,
                                                OP.mult, OP.add)
                        rn = vtile(f"rn3_{v}{i}")
                        nc.vector.tensor_mul(rn[:], r[:], t2[:])
                        r = rn
                    sc = vtile(f"sc_{v}")
                    nc.vector.tensor_scalar(sc[:], r[:], 7.5, None, OP.mult)
                    for i, (c, hh) in enumerate(((0, 0), (0, 1), (1, 0), (1, 1))):
                        eng = nc.vector
                        t = wnat[(v, c, hh, "t")]
                        sh = list(t.shape)
                        z = wqp.tile(sh, F32, name=f"z{v}{c}{hh}", tag="wz",
                                     bufs=4, padded_shape=[P, 5, 256])
                        tf = t.rearrange("p a b -> p (a b)")
                        zf = z.rearrange("p a b -> p (a b)")
                        eng.tensor_scalar(zf[:], tf[:], sc[:], 7.5, OP.mult, OP.add)
                        ri = wqp.tile(sh, I8, name=f"ri{v}{c}{hh}", tag="wr",
                                      bufs=4, padded_shape=[P, 5, 256])
                        rf = ri.rearrange("p a b -> p (a b)")
                        eng.tensor_scalar(rf[:], zf[:], 0.0, 15.0, OP.max, OP.min)
                        t0 = 0 if hh == 0 else 4
                        wTf = wT[v][c].rearrange("p a b -> p (a b)")
                        dst = wTf[:, t0 * 256:(t0 + sh[1]) * 256]
                        eng.tensor_scalar(dst, rf[:], 2.0, -15.0, OP.mult, OP.add)

            # ---------- BN0 stats over x (streamed) ----------
            with nc.named_scope("stats0"):
                for c in range(2):
                    for img in range(IMG):
                        for blk in range(4):
                            h0 = blk * 14
                            xt = qtp.tile([P, 14, H], F32, name=f"sx{c}{img}{blk}",
                                          tag="qin", bufs=6)
                            nc.sync.dma_start(
                                xt[:], xv[c * P:(c + 1) * P, img, h0:h0 + 14, :])
                            fl = xt.rearrange("p a b -> p (a b)")
                            k = (img * 4 + blk) * 2
                            nc.vector.bn_stats(xbn[c][:, k, :], fl[:, 0:392])
                            nc.vector.bn_stats(xbn[c][:, k + 1, :], fl[:, 392:784])
                    mv = stp.tile([P, 2], F32, name=f"mv0_{c}")
                    nc.vector.bn_aggr(mv[:], xbn[c][:, 0:16, :])
                    # local sums: n*(mean, var+mean^2), n = 6272
                    nc.vector.tensor_scalar(pk[0][:, 2 * c:2 * c + 1],
                                            mv[:, 0:1], 6272.0, None, OP.mult)
                    m2 = vtile(f"xm2_{c}")
                    nc.vector.tensor_mul(m2[:], mv[:, 0:1], mv[:, 0:1])
                    vp = vtile(f"xvp_{c}")
                    nc.vector.tensor_add(vp[:], mv[:, 1:2], m2[:])
                    nc.vector.tensor_scalar(pk[0][:, 2 * c + 1:2 * c + 2],
                                            vp[:], 6272.0, None, OP.mult)
                nc.sync.dma_start(ar_in[0][:], pk[0][:])
                nc.gpsimd.collective_compute(
                    "AllReduce", OP.add, replica_groups=[list(range(N_CORES))],
                    ins=[ar_in[0].opt()], outs=[ar_out[0].opt()])
                nc.sync.dma_start(gpk[0][:], ar_out[0][:])
                for c in range(2):
                    bn_coeffs(0, c, gpk[0][:, 2 * c:2 * c + 1],
                              gpk[0][:, 2 * c + 1:2 * c + 2], None)

            # ---------- quantize: z = a*in + b (ACT), clip->int8, ->fp8 ----------
            def quantize_block(src_ap, dst_ap, a, b, eng, names):
                z = qtp.tile([P, 14 * H], F32, name=names + "z", tag="qz", bufs=3)
                nc.scalar.activation(z[:], src_ap, AF.Identity,
                                     bias=b[:], scale=a[:])
                u = qtp.tile([P, 14 * H], I8, name=names + "u", tag="qu", bufs=3)
                eng.tensor_scalar(u[:], z[:], 0.0, 15.0, OP.max, OP.min)
                eng.tensor_copy(dst_ap, u[:])

            with nc.named_scope("quant0"):
                for c in range(2):
                    for img in range(IMG):
                        for blk in range(4):
                            h0 = blk * 14
                            xt = qtp.tile([P, 14, H], F32, name=f"qx{c}{img}{blk}",
                                          tag="qin", bufs=6)
                            nc.sync.dma_start(
                                xt[:], xv[c * P:(c + 1) * P, img, h0:h0 + 14, :])
                            lr = img * 58 + 1 + h0   # logical row
                            a0f = act0[c].rearrange("p r c -> p (r c)")
                            dst = a0f[:, lr * H:(lr + 14) * H]
                            quantize_block(xt.rearrange("p a b -> p (a b)")[:],
                                           dst, avec[0][c], bbvec[0][c],
                                           nc.vector, f"q0_{c}{img}{blk}")

            # ---------- conv (shared) ----------
            def conv(v, acts, epilogue):
                aflat = [acts[c].rearrange("p r c -> p (r c)") for c in range(2)]
                for gi, grp in enumerate(GROUPS):
                    for co in range(2):
                        psums = []
                        for wi, (r0, nr) in enumerate(grp):
                            ps = psp.tile([P, nr, H], F32,
                                          name=f"ps{v}_{gi}_{co}_{wi}",
                                          tag="psw", padded_shape=[P, 9, H])
                            psums.append(ps)
                        for ti, (dy, dx) in enumerate(TAPS):
                            tap = dy * 3 + dx
                            wlo, whi = max(0, 1 - dx), min(H, H + 1 - dx)
                            jlo = max(0, dx - 1)
                            for ci in range(2):
                                lhsT = wT[v][ci][:, tap, co * P:(co + 1) * P]
                                first = ti == 0 and ci == 0
                                last = ti == 8 and ci == 1
                                for wi, (r0, nr) in enumerate(grp):
                                    if dx == 1:
                                        rr = (r0 + dy - 1) * H
                                        rhs = aflat[ci][:, rr:rr + nr * H]
                                        out = psums[wi].rearrange(
                                            "p r c -> p (r c)")[:, 0:nr * H]
                                    else:
                                        rhs = acts[ci][:, r0 + dy - 1:
                                                       r0 + dy - 1 + nr,
                                                       jlo:jlo + whi - wlo]
                                        out = psums[wi][:, :, wlo:whi]
                                    nc.tensor.matmul(out, lhsT, rhs,
                                                     start=first, stop=last)
                        for wi, (r0, nr) in enumerate(grp):
                            epilogue(co, r0, nr, psums[wi])

            # ---------- conv0 + epilogue (spill S + interior sums) ----------
            slot_idx = [0, 0]

            def epi0(co, r0, nr, ps):
                psf = ps.rearrange("p r c -> p (r c)")
                for (rl, n, img, h0) in _runs(r0, nr):
                    sl = psf[:, (rl - r0) * H:(rl - r0 + n) * H]
                    tmp = runp.tile([P, n * H], F32, name=f"s_{co}_{rl}",
                                    tag="srun", bufs=6, padded_shape=[P, 9 * H])
                    k = slot_idx[co]
                    slot_idx[co] += 1
                    nc.scalar.activation(tmp[:], sl, AF.Identity,
                                         accum_out=sumS[co][:, k:k + 1])
                    sq = runp.tile([P, n * H], F32, name=f"sq_{co}_{rl}",
                                   tag="sq", bufs=4, padded_shape=[P, 9 * H])
                    nc.vector.scalar_tensor_tensor(
                        sq[:], tmp[:], 0.0, tmp[:], OP.bypass, OP.mult,
                        accum_out=sqS[co][:, k:k + 1])
                    nc.sync.dma_start(s_dram[co][:, img, h0:h0 + n, :], tmp[:])

            with nc.named_scope("conv0"):
                conv(0, act0, epi0)

            # ---------- BN1 stats + AR + coeffs ----------
            with nc.named_scope("stats1"):
                for c in range(2):
                    ns = slot_idx[c]
                    nc.vector.tensor_reduce(pk[1][:, 2 * c:2 * c + 1],
                                            sumS[c][:, 0:ns], AX.X, OP.add)
                    nc.vector.tensor_reduce(pk[1][:, 2 * c + 1:2 * c + 2],
                                            sqS[c][:, 0:ns], AX.X, OP.add)
                nc.sync.dma_start(ar_in[1][:], pk[1][:])
                nc.gpsimd.collective_compute(
                    "AllReduce", OP.add, replica_groups=[list(range(N_CORES))],
                    ins=[ar_in[1].opt()], outs=[ar_out[1].opt()])
                nc.sync.dma_start(gpk[1][:], ar_out[1][:])
                for c in range(2):
                    bn_coeffs(1, c, gpk[1][:, 2 * c:2 * c + 1],
                              gpk[1][:, 2 * c + 1:2 * c + 2], svec[0])

            # ---------- quantize1: S -> act1 codes ----------
            with nc.named_scope("quant1"):
                for c in range(2):
                    for img in range(IMG):
                        for blk in range(4):
                            h0 = blk * 14
                            st = qtp.tile([P, 14, H], F32, name=f"qs{c}{img}{blk}",
                                          tag="qin", bufs=6)
                            nc.sync.dma_start(
                                st[:], s_dram[c][:, img, h0:h0 + 14, :])
                            lr = img * 58 + 1 + h0
                            a1f = act1[c].rearrange("p r c -> p (r c)")
                            dst = a1f[:, lr * H:(lr + 14) * H]
                            quantize_block(st.rearrange("p a b -> p (a b)")[:],
                                           dst, avec[1][c], bbvec[1][c],
                                           nc.vector, f"q1_{c}{img}{blk}")

            # ---------- conv1 + residual epilogue ----------
            def epi1(co, r0, nr, ps):
                psf = ps.rearrange("p r c -> p (r c)")
                for (rl, n, img, h0) in _runs(r0, nr):
                    sl = psf[:, (rl - r0) * H:(rl - r0 + n) * H]
                    xt = runp.tile([P, n * H], F32, name=f"x_{co}_{rl}",
                                   tag="xrun", bufs=6, padded_shape=[P, 9 * H])
                    nc.sync.dma_start(
                        xt[:], xv[co * P:(co + 1) * P, img, h0:h0 + n, :])
                    ot = runp.tile([P, n * H], F32, name=f"o_{co}_{rl}",
                                   tag="srun", bufs=6, padded_shape=[P, 9 * H])
                    nc.vector.scalar_tensor_tensor(
                        ot[:], sl, svec[1][:], xt[:], OP.mult, OP.add)
                    nc.sync.dma_start(
                        ov[co * P:(co + 1) * P, img, h0:h0 + n, :],
                        ot.rearrange("p (a b) -> p a b", b=H)[:])

            with nc.named_scope("conv1"):
                conv(1, act1, epi1)

    nc.compile()
    return nc


def _install_ntff_hook():
    """Provide antenv.axon_hooks (absent in this image) via ctypes so that
    run_bass_kernel_spmd(trace=True) can capture NTFF profiles."""
    try:
        from antenv.axon_hooks import get_axon_ntff_profile_hook  # noqa: F401
        return
    except ImportError:
        pass
    import contextlib
    import ctypes
    import types

    so_path = "/opt/axon/libaxon_pjrt.so"
    if not os.path.exists(so_path):
        return
    lib = ctypes.CDLL(so_path)
    if not hasattr(lib, "axon_start_nrt_profile"):
        return
    lib.axon_start_nrt_profile.argtypes = [ctypes.POINTER(ctypes.c_int64),
                                           ctypes.c_size_t]
    lib.axon_start_nrt_profile.restype = ctypes.c_int64
    lib.axon_stop_nrt_profile.argtypes = [ctypes.c_char_p]
    lib.axon_stop_nrt_profile.restype = ctypes.c_int64

    @contextlib.contextmanager
    def _hook(output_dir, device_ids):
        import jax
        jax.devices()
        if device_ids:
            ids = (ctypes.c_int64 * len(device_ids))(*device_ids)
            rc = lib.axon_start_nrt_profile(ids, len(device_ids))
        else:
            rc = lib.axon_start_nrt_profile(None, 0)
        if rc != 0:
            raise RuntimeError(f"axon_start_nrt_profile rc={rc}")
        try:
            yield
        finally:
            n = lib.axon_stop_nrt_profile(str(output_dir).encode())
            print(f"ntff profile: {n} file(s) written to {output_dir}")

    hook_holder = [_hook]
    mod = types.ModuleType("antenv.axon_hooks")
    mod.get_axon_ntff_profile_hook = lambda: hook_holder[0]
    mod.set_axon_ntff_profile_hook = lambda h: hook_holder.__setitem__(0, h)
    import antenv
    sys.modules["antenv.axon_hooks"] = mod
    antenv.axon_hooks = mod


_NC = None


def _get_nc():
    global _NC
    if _NC is None:
        _NC = build()
    return _NC


LAST_RESULTS = None


def kernel(x, bn0_gamma, bn0_beta, conv0_w, bn1_gamma, bn1_beta, conv1_w):
    global LAST_RESULTS
    nc = _get_nc()
    shared = {
        # permute OIHW -> [i, kh, kw, o] so on-chip weight access is contiguous
        "conv0_w": np.ascontiguousarray(
            np.asarray(conv0_w, np.float32).transpose(1, 2, 3, 0)),
        "conv1_w": np.ascontiguousarray(
            np.asarray(conv1_w, np.float32).transpose(1, 2, 3, 0)),
        "bn0_gamma": np.ascontiguousarray(bn0_gamma, np.float32),
        "bn0_beta": np.ascontiguousarray(bn0_beta, np.float32),
        "bn1_gamma": np.ascontiguousarray(bn1_gamma, np.float32),
        "bn1_beta": np.ascontiguousarray(bn1_beta, np.float32),
    }
    x = np.ascontiguousarray(x, np.float32)
    in_maps = [{"x": x[2 * c:2 * c + 2], **shared} for c in range(N_CORES)]
    trace = bool(int(os.environ.get("KERNEL_TRACE", "0")))
    if trace:
        _install_ntff_hook()
    res = bass_utils.run_bass_kernel_spmd(
        nc, in_maps, core_ids=list(range(N_CORES)), trace=trace)
    LAST_RESULTS = res
    return np.concatenate([res.results[c]["out"] for c in range(N_CORES)], axis=0)


# revision 8
# speedup vs baseline: 2.7532x; 1.3630x over previous
"""Trainium2 Bass kernel: PreActBlock with DoReFa 4-bit quantization (sync-BN).

  out = conv3x3(q(relu(BN1(conv3x3(q(relu(BN0(x))), qw(w0))))), qw(w1)) + x

Design (8 cores, data-parallel over batch 16 -> 2 images/core):
 - Quantized activations are integers 0..15 and quantized weights odd integers
   -15..15 (x scale).  Both are exact in fp8e4 (e4m3) and the PE accumulates
   in fp32, so every conv is computed EXACTLY as integer sums (|S| < 2^20).
 - fp8 DoubleRow matmuls: contraction K=256 per instruction via the
   [P, 2, ...] interleaved layout (2x PE throughput).
 - BN batch stats are all-reduced across the 8 cores (sync-BN semantics).
 - Rounding: fp32->int8 cast is round-to-nearest-even (HW-validated), so
   quantize = clip(tensor_scalar max/min -> int8) + int8->fp8 copy.
 - Spatial layout: unpadded 56-wide rows; 2 images stacked with zero pad rows
   (116 rows).  Column zero-padding is implicit: edge taps run one column
   narrower (the skipped outputs receive exactly the zero-pad contribution).
 - Conv: per 9-row window (N = 504 <= 512, one PSUM bank) accumulate 9
   DoubleRow taps; weights host-permuted to [i, kh, kw, o] so weight DMA and
   quantized code writes are contiguous.
"""
import os
import sys

sys.path.insert(0, "/opt/trn_rl_repo")

import numpy as np

import concourse.bacc as bacc
import concourse.bass as bass
import concourse.mybir as mybir
from concourse import tile
from concourse import bass_utils

F32 = mybir.dt.float32
FP8 = mybir.dt.float8e4
I8 = mybir.dt.int8
AX = mybir.AxisListType
OP = mybir.AluOpType
AF = mybir.ActivationFunctionType
PM = mybir.MatmulPerfMode

P = 128
N_CORES = 8
IMG = 2              # images per core
H = 56
ROWS = 116           # 2 images x (1 pad + 56 + 1 pad) rows
CNT = 50176.0        # global BN count: 16 * 56 * 56
EPS = 1e-5

# 9-row output windows over logical rows 1..114
WINDOWS = [(1 + 9 * k, 9) for k in range(12)] + [(109, 6)]
GROUPS = [WINDOWS[0:4], WINDOWS[4:8], WINDOWS[8:12], WINDOWS[12:13]]
# tap order: full-width tap (dy=0,dx=1) first so start=True covers all columns
TAPS = [(0, 1), (0, 0), (0, 2), (1, 0), (1, 1), (1, 2), (2, 0), (2, 1), (2, 2)]


def _runs(r0, nr):
    """Interior row-runs of a window: (logical_row, nrows, img, h0)."""
    out = []
    for lo, hi, img, base in ((1, 56, 0, 1), (59, 114, 1, 59)):
        a, b = max(r0, lo), min(r0 + nr - 1, hi)
        if a <= b:
            out.append((a, b - a + 1, img, a - base))
    return out


def build():
    nc = bacc.Bacc("TRN2", target_bir_lowering=False, debug=False,
                   enable_asserts=False, num_devices=N_CORES)

    x_d = nc.dram_tensor("x", [IMG, 256, H, H], F32, kind="ExternalInput")
    # host-permuted to [i, kh, kw, o]
    w_d = [nc.dram_tensor("conv0_w", [256, 3, 3, 256], F32, kind="ExternalInput"),
           nc.dram_tensor("conv1_w", [256, 3, 3, 256], F32, kind="ExternalInput")]
    g_d = [nc.dram_tensor("bn0_gamma", [256], F32, kind="ExternalInput"),
           nc.dram_tensor("bn1_gamma", [256], F32, kind="ExternalInput")]
    b_d = [nc.dram_tensor("bn0_beta", [256], F32, kind="ExternalInput"),
           nc.dram_tensor("bn1_beta", [256], F32, kind="ExternalInput")]
    out_d = nc.dram_tensor("out", [IMG, 256, H, H], F32, kind="ExternalOutput")

    xv = x_d.ap().rearrange("n c h w -> c n h w")       # [256, 2, 56, 56]
    ov = out_d.ap().rearrange("n c h w -> c n h w")

    with tile.TileContext(nc) as tc:
        with tc.tile_pool(name="act", bufs=1) as actp, \
             tc.tile_pool(name="wtp", bufs=1) as wtp, \
             tc.tile_pool(name="wq", bufs=4) as wqp, \
             tc.tile_pool(name="qt", bufs=3) as qtp, \
             tc.tile_pool(name="run", bufs=6) as runp, \
             tc.tile_pool(name="st", bufs=1) as stp, \
             tc.tile_pool(name="ps", bufs=8, space="PSUM") as psp, \
             tc.tile_pool(name="dram", bufs=1, space="DRAM") as drp:

            # ---------- static tiles ----------
            # interleaved ci-chunk layout for DoubleRow: [P, ki, rows, col]
            act0 = actp.tile([P, 2, ROWS, H], FP8, name="act0")
            act1 = actp.tile([P, 2, ROWS, H], FP8, name="act1")
            # quantized weight codes, [ci_lo, tap, ki, co] fp8
            wT = [wtp.tile([P, 9, 2, 256], FP8, name=f"w{v}T") for v in range(2)]
            # DRAM spill of conv0 integer outputs (unpadded interior)
            s_dram = [drp.tile([P, IMG, H, H], F32, name=f"s_dram_{c}")
                      for c in range(2)]
            ar_in = [drp.tile([P, 4], F32, name=f"ar_in_{i}") for i in range(2)]
            ar_out = [drp.tile([P, 4], F32, name=f"ar_out_{i}") for i in range(2)]

            # stats / small vectors
            xbn = [stp.tile([P, 16, 6], F32, name=f"xbn_{c}") for c in range(2)]
            sumS = [stp.tile([P, 16], F32, name=f"sumS_{c}") for c in range(2)]
            sqS = [stp.tile([P, 16], F32, name=f"sqS_{c}") for c in range(2)]
            gvec = [[stp.tile([P, 1], F32, name=f"g{v}_{c}") for c in range(2)]
                    for v in range(2)]
            bvec = [[stp.tile([P, 1], F32, name=f"b{v}_{c}") for c in range(2)]
                    for v in range(2)]
            avec = [[stp.tile([P, 1], F32, name=f"a{v}_{c}") for c in range(2)]
                    for v in range(2)]
            bbvec = [[stp.tile([P, 1], F32, name=f"bb{v}_{c}") for c in range(2)]
                     for v in range(2)]
            svec = [stp.tile([P, 1], F32, name=f"scale_{v}") for v in range(2)]
            pk = [stp.tile([P, 4], F32, name=f"pk_{i}") for i in range(2)]
            gpk = [stp.tile([P, 4], F32, name=f"gpk_{i}") for i in range(2)]

            def vtile(name):
                return stp.tile([P, 1], F32, name=name, tag="vtmp", bufs=8)

            # ---------- tiny vector helpers (all on [P,1]) ----------
            def refined_rsqrt(vpe, name):
                """rsqrt(vpe) with 2 Newton refinements (vpe > 0)."""
                r = vtile(f"{name}_r")
                nc.vector.reciprocal(r[:], vpe[:])
                y = vtile(f"{name}_y")
                nc.scalar.activation(y[:], r[:], AF.Sqrt)
                for i in range(2):
                    y2 = vtile(f"{name}_y2{i}")
                    nc.vector.tensor_mul(y2[:], y[:], y[:])
                    t2 = vtile(f"{name}_t2{i}")
                    nc.vector.tensor_mul(t2[:], vpe[:], y2[:])
                    h = vtile(f"{name}_h{i}")
                    nc.vector.tensor_scalar(h[:], t2[:], -0.5, 1.5, OP.mult, OP.add)
                    yn = vtile(f"{name}_yn{i}")
                    nc.vector.tensor_mul(yn[:], y[:], h[:])
                    y = yn
                return y

            def bn_coeffs(v, c, gsum, gsumsq, scale):
                """a,b for z = a*S + b  (= 15 * BN-affine), scale=None for BN0."""
                mean = vtile(f"m{v}{c}")
                nc.vector.tensor_scalar(mean[:], gsum, 1.0 / CNT, None, OP.mult)
                ex2 = vtile(f"e{v}{c}")
                nc.vector.tensor_scalar(ex2[:], gsumsq, 1.0 / CNT, None, OP.mult)
                m2 = vtile(f"m2{v}{c}")
                nc.vector.tensor_mul(m2[:], mean[:], mean[:])
                var = vtile(f"va{v}{c}")
                nc.vector.tensor_sub(var[:], ex2[:], m2[:])
                if scale is not None:
                    s2 = vtile(f"s2{v}{c}")
                    nc.vector.tensor_mul(s2[:], scale[:], scale[:])
                    nc.vector.tensor_mul(var[:], var[:], s2[:])
                    mo = vtile(f"mo{v}{c}")
                    nc.vector.tensor_mul(mo[:], mean[:], scale[:])
                    mean = mo
                vpe = vtile(f"vp{v}{c}")
                nc.vector.tensor_scalar(vpe[:], var[:], EPS, None, OP.add)
                rs = refined_rsqrt(vpe, f"rs{v}{c}")
                grs = vtile(f"gr{v}{c}")
                nc.vector.tensor_mul(grs[:], gvec[v][c][:], rs[:])
                a = avec[v][c]
                if scale is not None:
                    asc = vtile(f"as{v}{c}")
                    nc.vector.tensor_mul(asc[:], grs[:], scale[:])
                    nc.vector.tensor_scalar(a[:], asc[:], 15.0, None, OP.mult)
                else:
                    nc.vector.tensor_scalar(a[:], grs[:], 15.0, None, OP.mult)
                mg = vtile(f"mg{v}{c}")
                nc.vector.tensor_mul(mg[:], mean[:], grs[:])
                mg15 = vtile(f"mh{v}{c}")
                nc.vector.tensor_scalar(mg15[:], mg[:], 15.0, None, OP.mult)
                b15 = vtile(f"bh{v}{c}")
                nc.vector.tensor_scalar(b15[:], bvec[v][c][:], 15.0, None, OP.mult)
                nc.vector.tensor_sub(bbvec[v][c][:], b15[:], mg15[:])

            # ---------- load BN params ----------
            for v in range(2):
                for c in range(2):
                    nc.sync.dma_start(gvec[v][c][:], g_d[v].ap()[c * P:(c + 1) * P])
                    nc.sync.dma_start(bvec[v][c][:], b_d[v].ap()[c * P:(c + 1) * P])

            # ---------- act pad-row zeroing ----------
            with nc.named_scope("memset"):
                for t in (act0, act1):
                    for k in range(2):
                        for r in (0, 57, 58, 115):
                            nc.gpsimd.memset(t[:, k, r, :], 0.0)

            # ---------- weight quantization ----------
            # DRAM layout [i, kh, kw, o] -> contiguous [ci_lo, tap, ki, co]
            def wquant(v):
                mxp = stp.tile([P, 4], F32, name=f"mxp_{v}")
                wv = w_d[v].ap().rearrange("i kh kw o -> i (kh kw) o")
                wnat = {}
                for ki in range(2):
                    for hh in range(2):  # tap halves: 0 -> taps 0..3, 1 -> 4..8
                        t0, t1 = (0, 4) if hh == 0 else (4, 9)
                        wn = wqp.tile([P, t1 - t0, 256], F32,
                                      name=f"wn{v}{ki}{hh}", tag="wnat", bufs=4,
                                      padded_shape=[P, 5, 256])
                        nc.sync.dma_start(
                            wn[:], wv[ki * P:(ki + 1) * P, t0:t1, :])
                        wnat[(ki, hh)] = wn
                for i, (ki, hh) in enumerate(((0, 0), (0, 1), (1, 0), (1, 1))):
                    wn = wnat[(ki, hh)]
                    t = wqp.tile(list(wn.shape), F32, name=f"t{v}{ki}{hh}",
                                 tag="tanh", bufs=4, padded_shape=[P, 5, 256])
                    tf = t.rearrange("p a b -> p (a b)")
                    wf = wn.rearrange("p a b -> p (a b)")
                    nc.scalar.activation(tf[:], wf[:], AF.Tanh)
                    nc.vector.tensor_reduce(
                        mxp[:, i:i + 1], tf[:], AX.X, OP.max,
                        apply_absolute_value=True)
                    wnat[(ki, hh, "t")] = t
                mx1 = vtile(f"mx1_{v}")
                nc.vector.tensor_reduce(mx1[:], mxp[:], AX.X, OP.max,
                                        apply_absolute_value=True)
                msc = stp.tile([1, 1], F32, name=f"msc_{v}")
                nc.gpsimd.tensor_reduce(msc[:], mx1[:], AX.C, OP.max)
                mvec = vtile(f"mvec_{v}")
                nc.gpsimd.partition_broadcast(mvec[:], msc[:])
                # svec = M/225 (psum scale); r = 7.5/M for codes
                nc.vector.tensor_scalar(svec[v][:], mvec[:], 1.0 / 225.0,
                                        None, OP.mult)
                r = vtile(f"rin_{v}")
                nc.vector.reciprocal(r[:], mvec[:])
                for i in range(2):  # Newton: r = r*(2 - M*r)
                    t1_ = vtile(f"rn1_{v}{i}")
                    nc.vector.tensor_mul(t1_[:], mvec[:], r[:])
                    t2_ = vtile(f"rn2_{v}{i}")
                    nc.vector.tensor_scalar(t2_[:], t1_[:], -1.0, 2.0,
                                            OP.mult, OP.add)
                    rn = vtile(f"rn3_{v}{i}")
                    nc.vector.tensor_mul(rn[:], r[:], t2_[:])
                    r = rn
                sc = vtile(f"sc_{v}")
                nc.vector.tensor_scalar(sc[:], r[:], 7.5, None, OP.mult)
                for i, (ki, hh) in enumerate(((0, 0), (0, 1), (1, 0), (1, 1))):
                    t = wnat[(ki, hh, "t")]
                    sh = list(t.shape)
                    tf = t.rearrange("p a b -> p (a b)")
                    z = wqp.tile(sh, F32, name=f"z{v}{ki}{hh}", tag="wz",
                                 bufs=4, padded_shape=[P, 5, 256])
                    zf = z.rearrange("p a b -> p (a b)")
                    nc.vector.tensor_scalar(zf[:], tf[:], sc[:], 7.5,
                                            OP.mult, OP.add)
                    ri = wqp.tile(sh, I8, name=f"ri{v}{ki}{hh}", tag="wr",
                                  bufs=4, padded_shape=[P, 5, 256])
                    rf = ri.rearrange("p a b -> p (a b)")
                    nc.vector.tensor_scalar(rf[:], zf[:], 0.0, 15.0,
                                            OP.max, OP.min)
                    t0 = 0 if hh == 0 else 4
                    dst = wT[v][:, t0:t0 + sh[1], ki, :]
                    nc.vector.tensor_scalar(dst, ri[:], 2.0, -15.0,
                                            OP.mult, OP.add)

            with nc.named_scope("wquant0"):
                wquant(0)

            # ---------- BN0 stats over x (streamed) ----------
            with nc.named_scope("stats0"):
                for c in range(2):
                    for img in range(IMG):
                        for blk in range(4):
                            h0 = blk * 14
                            xt = qtp.tile([P, 14, H], F32, name=f"sx{c}{img}{blk}",
                                          tag="qin", bufs=6)
                            nc.sync.dma_start(
                                xt[:], xv[c * P:(c + 1) * P, img, h0:h0 + 14, :])
                            fl = xt.rearrange("p a b -> p (a b)")
                            k = (img * 4 + blk) * 2
                            nc.vector.bn_stats(xbn[c][:, k, :], fl[:, 0:392])
                            nc.vector.bn_stats(xbn[c][:, k + 1, :], fl[:, 392:784])
                    mv = stp.tile([P, 2], F32, name=f"mv0_{c}")
                    nc.vector.bn_aggr(mv[:], xbn[c][:, 0:16, :])
                    # local sums: n*(mean, var+mean^2), n = 6272
                    nc.vector.tensor_scalar(pk[0][:, 2 * c:2 * c + 1],
                                            mv[:, 0:1], 6272.0, None, OP.mult)
                    m2 = vtile(f"xm2_{c}")
                    nc.vector.tensor_mul(m2[:], mv[:, 0:1], mv[:, 0:1])
                    vp = vtile(f"xvp_{c}")
                    nc.vector.tensor_add(vp[:], mv[:, 1:2], m2[:])
                    nc.vector.tensor_scalar(pk[0][:, 2 * c + 1:2 * c + 2],
                                            vp[:], 6272.0, None, OP.mult)
                nc.sync.dma_start(ar_in[0][:], pk[0][:])
                nc.gpsimd.collective_compute(
                    "AllReduce", OP.add, replica_groups=[list(range(N_CORES))],
                    ins=[ar_in[0].opt()], outs=[ar_out[0].opt()])
                nc.sync.dma_start(gpk[0][:], ar_out[0][:])
                for c in range(2):
                    bn_coeffs(0, c, gpk[0][:, 2 * c:2 * c + 1],
                              gpk[0][:, 2 * c + 1:2 * c + 2], None)

            # ---------- quantize: z = a*in + b (ACT), clip->int8, ->fp8 ----------
            def quantize_block(src_ap, dst_ap, a, b, names):
                z = qtp.tile([P, 14 * H], F32, name=names + "z", tag="qz", bufs=3)
                nc.scalar.activation(z[:], src_ap, AF.Identity,
                                     bias=b[:], scale=a[:])
                u = qtp.tile([P, 14 * H], I8, name=names + "u", tag="qu", bufs=3)
                nc.vector.tensor_scalar(u[:], z[:], 0.0, 15.0, OP.max, OP.min)
                nc.vector.tensor_copy(dst_ap, u[:])

            def act_dst(t, c, lr, n):
                af = t.rearrange("p k r c -> p (k r c)")
                o = (c * ROWS + lr) * H
                return af[:, o:o + n * H]

            with nc.named_scope("quant0"):
                for img in range(IMG):
                    for blk in range(4):
                        for c in range(2):
                            h0 = blk * 14
                            xt = qtp.tile([P, 14, H], F32, name=f"qx{c}{img}{blk}",
                                          tag="qin", bufs=6)
                            nc.scalar.dma_start(
                                xt[:], xv[c * P:(c + 1) * P, img, h0:h0 + 14, :])
                            lr = img * 58 + 1 + h0   # logical row
                            quantize_block(xt.rearrange("p a b -> p (a b)")[:],
                                           act_dst(act0, c, lr, 14),
                                           avec[0][c], bbvec[0][c],
                                           f"q0_{c}{img}{blk}")

            # ---------- conv (shared), fp8 DoubleRow, K=256 per matmul ----------
            def conv(v, act, epilogue):
                for gi, grp in enumerate(GROUPS):
                    for co in range(2):
                        psums = []
                        for wi, (r0, nr) in enumerate(grp):
                            ps = psp.tile([P, nr, H], F32,
                                          name=f"ps{v}_{gi}_{co}_{wi}",
                                          tag="psw", padded_shape=[P, 9, H])
                            psums.append(ps)
                        for ti, (dy, dx) in enumerate(TAPS):
                            tap = dy * 3 + dx
                            wlo, whi = max(0, 1 - dx), min(H, H + 1 - dx)
                            jlo = max(0, dx - 1)
                            lhsT = wT[v][:, tap, :, co * P:(co + 1) * P]
                            first = ti == 0
                            last = ti == 8
                            for wi, (r0, nr) in enumerate(grp):
                                rows = slice(r0 + dy - 1, r0 + dy - 1 + nr)
                                if dx == 1:
                                    rhs = act[:, :, rows, :].rearrange(
                                        "p k r c -> p k (r c)")
                                    out = psums[wi].rearrange(
                                        "p r c -> p (r c)")[:, 0:nr * H]
                                else:
                                    rhs = act[:, :, rows, jlo:jlo + whi - wlo]
                                    out = psums[wi][:, :, wlo:whi]
                                nc.tensor.matmul(out, lhsT, rhs,
                                                 start=first, stop=last,
                                                 perf_mode=PM.DoubleRow)
                        for wi, (r0, nr) in enumerate(grp):
                            epilogue(co, r0, nr, psums[wi])

            # ---------- conv0 + epilogue (spill S + interior sums) ----------
            slot_idx = [0, 0]

            def epi0(co, r0, nr, ps):
                psf = ps.rearrange("p r c -> p (r c)")
                for (rl, n, img, h0) in _runs(r0, nr):
                    sl = psf[:, (rl - r0) * H:(rl - r0 + n) * H]
                    tmp = runp.tile([P, n * H], F32, name=f"s_{co}_{rl}",
                                    tag="srun", bufs=6, padded_shape=[P, 9 * H])
                    k = slot_idx[co]
                    slot_idx[co] += 1
                    nc.scalar.activation(tmp[:], sl, AF.Identity,
                                         accum_out=sumS[co][:, k:k + 1])
                    sq = runp.tile([P, n * H], F32, name=f"sq_{co}_{rl}",
                                   tag="sq", bufs=4, padded_shape=[P, 9 * H])
                    nc.vector.scalar_tensor_tensor(
                        sq[:], tmp[:], 0.0, tmp[:], OP.bypass, OP.mult,
                        accum_out=sqS[co][:, k:k + 1])
                    nc.sync.dma_start(s_dram[co][:, img, h0:h0 + n, :], tmp[:])

            with nc.named_scope("conv0"):
                conv(0, act0, epi0)

            with nc.named_scope("wquant1"):
                wquant(1)

            # ---------- BN1 stats + AR + coeffs ----------
            with nc.named_scope("stats1"):
                for c in range(2):
                    ns = slot_idx[c]
                    nc.vector.tensor_reduce(pk[1][:, 2 * c:2 * c + 1],
                                            sumS[c][:, 0:ns], AX.X, OP.add)
                    nc.vector.tensor_reduce(pk[1][:, 2 * c + 1:2 * c + 2],
                                            sqS[c][:, 0:ns], AX.X, OP.add)
                nc.sync.dma_start(ar_in[1][:], pk[1][:])
                nc.gpsimd.collective_compute(
                    "AllReduce", OP.add, replica_groups=[list(range(N_CORES))],
                    ins=[ar_in[1].opt()], outs=[ar_out[1].opt()])
                nc.sync.dma_start(gpk[1][:], ar_out[1][:])
                for c in range(2):
                    bn_coeffs(1, c, gpk[1][:, 2 * c:2 * c + 1],
                              gpk[1][:, 2 * c + 1:2 * c + 2], svec[0])

            # ---------- quantize1: S -> act1 codes ----------
            with nc.named_scope("quant1"):
                for img in range(IMG):
                    for blk in range(4):
                        for c in range(2):
                            h0 = blk * 14
                            st = qtp.tile([P, 14, H], F32, name=f"qs{c}{img}{blk}",
                                          tag="qin", bufs=6)
                            nc.scalar.dma_start(
                                st[:], s_dram[c][:, img, h0:h0 + 14, :])
                            lr = img * 58 + 1 + h0
                            quantize_block(st.rearrange("p a b -> p (a b)")[:],
                                           act_dst(act1, c, lr, 14),
                                           avec[1][c], bbvec[1][c],
                                           f"q1_{c}{img}{blk}")

            # ---------- conv1 + residual epilogue ----------
            def epi1(co, r0, nr, ps):
                psf = ps.rearrange("p r c -> p (r c)")
                for (rl, n, img, h0) in _runs(r0, nr):
                    sl = psf[:, (rl - r0) * H:(rl - r0 + n) * H]
                    xt = runp.tile([P, n * H], F32, name=f"x_{co}_{rl}",
                                   tag="xrun", bufs=6, padded_shape=[P, 9 * H])
                    nc.sync.dma_start(
                        xt[:], xv[co * P:(co + 1) * P, img, h0:h0 + n, :])
                    ot = runp.tile([P, n * H], F32, name=f"o_{co}_{rl}",
                                   tag="srun", bufs=6, padded_shape=[P, 9 * H])
                    nc.vector.scalar_tensor_tensor(
                        ot[:], sl, svec[1][:], xt[:], OP.mult, OP.add)
                    nc.sync.dma_start(
                        ov[co * P:(co + 1) * P, img, h0:h0 + n, :],
                        ot.rearrange("p (a b) -> p a b", b=H)[:])

            with nc.named_scope("conv1"):
                conv(1, act1, epi1)

    nc.compile()
    return nc


def _install_ntff_hook():
    """Provide antenv.axon_hooks (absent in this image) via ctypes so that
    run_bass_kernel_spmd(trace=True) can capture NTFF profiles."""
    try:
        from antenv.axon_hooks import get_axon_ntff_profile_hook  # noqa: F401
        return
    except ImportError:
        pass
    import contextlib
    import ctypes
    import types

    so_path = "/opt/axon/libaxon_pjrt.so"
    if not os.path.exists(so_path):
        return
    lib = ctypes.CDLL(so_path)
    if not hasattr(lib, "axon_start_nrt_profile"):
        return
    lib.axon_start_nrt_profile.argtypes = [ctypes.POINTER(ctypes.c_int64),
                                           ctypes.c_size_t]
    lib.axon_start_nrt_profile.restype = ctypes.c_int64
    lib.axon_stop_nrt_profile.argtypes = [ctypes.c_char_p]
    lib.axon_stop_nrt_profile.restype = ctypes.c_int64

    @contextlib.contextmanager
    def _hook(output_dir, device_ids):
        import jax
        jax.devices()
        if device_ids:
            ids = (ctypes.c_int64 * len(device_ids))(*device_ids)
            rc = lib.axon_start_nrt_profile(ids, len(device_ids))
        else:
            rc = lib.axon_start_nrt_profile(None, 0)
        if rc != 0:
            raise RuntimeError(f"axon_start_nrt_profile rc={rc}")
        try:
            yield
        finally:
            n = lib.axon_stop_nrt_profile(str(output_dir).encode())
            print(f"ntff profile: {n} file(s) written to {output_dir}")

    hook_holder = [_hook]
    mod = types.ModuleType("antenv.axon_hooks")
    mod.get_axon_ntff_profile_hook = lambda: hook_holder[0]
    mod.set_axon_ntff_profile_hook = lambda h: hook_holder.__setitem__(0, h)
    import antenv
    sys.modules["antenv.axon_hooks"] = mod
    antenv.axon_hooks = mod


_NC = None


def _get_nc():
    global _NC
    if _NC is None:
        _NC = build()
    return _NC


LAST_RESULTS = None


def kernel(x, bn0_gamma, bn0_beta, conv0_w, bn1_gamma, bn1_beta, conv1_w):
    global LAST_RESULTS
    nc = _get_nc()
    shared = {
        # permute OIHW -> [i, kh, kw, o] so on-chip weight access is contiguous
        "conv0_w": np.ascontiguousarray(
            np.asarray(conv0_w, np.float32).transpose(1, 2, 3, 0)),
        "conv1_w": np.ascontiguousarray(
            np.asarray(conv1_w, np.float32).transpose(1, 2, 3, 0)),
        "bn0_gamma": np.ascontiguousarray(bn0_gamma, np.float32),
        "bn0_beta": np.ascontiguousarray(bn0_beta, np.float32),
        "bn1_gamma": np.ascontiguousarray(bn1_gamma, np.float32),
        "bn1_beta": np.ascontiguousarray(bn1_beta, np.float32),
    }
    x = np.ascontiguousarray(x, np.float32)
    in_maps = [{"x": x[2 * c:2 * c + 2], **shared} for c in range(N_CORES)]
    trace = bool(int(os.environ.get("KERNEL_TRACE", "0")))
    if trace:
        _install_ntff_hook()
    res = bass_utils.run_bass_kernel_spmd(
        nc, in_maps, core_ids=list(range(N_CORES)), trace=trace)
    LAST_RESULTS = res
    return np.concatenate([res.results[c]["out"] for c in range(N_CORES)], axis=0)


# revision 9
# speedup vs baseline: 2.8481x; 1.0345x over previous
"""Trainium2 Bass kernel: PreActBlock with DoReFa 4-bit quantization (sync-BN).

  out = conv3x3(q(relu(BN1(conv3x3(q(relu(BN0(x))), qw(w0))))), qw(w1)) + x

Design (8 cores, data-parallel over batch 16 -> 2 images/core):
 - Quantized activations are integers 0..15 and quantized weights odd integers
   -15..15 (x scale).  Both are exact in fp8e4 (e4m3) and the PE accumulates
   in fp32, so every conv is computed EXACTLY as integer sums (|S| < 2^20).
 - fp8 DoubleRow matmuls: contraction K=256 per instruction via the
   [P, 2, ...] interleaved layout (2x PE throughput).
 - BN batch stats are all-reduced across the 8 cores (sync-BN semantics).
 - Rounding: fp32->int8 cast is round-to-nearest-even (HW-validated), so
   quantize = clip(tensor_scalar max/min -> int8) + int8->fp8 copy.
 - Spatial layout: unpadded 56-wide rows; 2 images stacked with zero pad rows
   (116 rows).  Column zero-padding is implicit: edge taps run one column
   narrower (the skipped outputs receive exactly the zero-pad contribution).
 - Conv: per 9-row window (N = 504 <= 512, one PSUM bank) accumulate 9
   DoubleRow taps; weights host-permuted to [i, kh, kw, o] so weight DMA and
   quantized code writes are contiguous.
"""
import os
import sys

sys.path.insert(0, "/opt/trn_rl_repo")

import numpy as np

import concourse.bacc as bacc
import concourse.bass as bass
import concourse.mybir as mybir
from concourse import tile
from concourse import bass_utils

F32 = mybir.dt.float32
FP8 = mybir.dt.float8e4
I8 = mybir.dt.int8
AX = mybir.AxisListType
OP = mybir.AluOpType
AF = mybir.ActivationFunctionType
PM = mybir.MatmulPerfMode

P = 128
N_CORES = 8
IMG = 2              # images per core
H = 56
ROWS = 116           # 2 images x (1 pad + 56 + 1 pad) rows
CNT = 50176.0        # global BN count: 16 * 56 * 56
EPS = 1e-5

# 9-row output windows over logical rows 1..114
WINDOWS = [(1 + 9 * k, 9) for k in range(12)] + [(109, 6)]
GROUPS = [WINDOWS[0:4], WINDOWS[4:8], WINDOWS[8:12], WINDOWS[12:13]]
# tap order: full-width tap (dy=0,dx=1) first so start=True covers all columns
TAPS = [(0, 1), (0, 0), (0, 2), (1, 0), (1, 1), (1, 2), (2, 0), (2, 1), (2, 2)]


def _runs(r0, nr):
    """Interior row-runs of a window: (logical_row, nrows, img, h0)."""
    out = []
    for lo, hi, img, base in ((1, 56, 0, 1), (59, 114, 1, 59)):
        a, b = max(r0, lo), min(r0 + nr - 1, hi)
        if a <= b:
            out.append((a, b - a + 1, img, a - base))
    return out


def build():
    nc = bacc.Bacc("TRN2", target_bir_lowering=False, debug=False,
                   enable_asserts=False, num_devices=N_CORES)

    x_d = nc.dram_tensor("x", [IMG, 256, H, H], F32, kind="ExternalInput")
    # host-permuted to [i, kh, kw, o]
    w_d = [nc.dram_tensor("conv0_w", [256, 3, 3, 256], F32, kind="ExternalInput"),
           nc.dram_tensor("conv1_w", [256, 3, 3, 256], F32, kind="ExternalInput")]
    g_d = [nc.dram_tensor("bn0_gamma", [256], F32, kind="ExternalInput"),
           nc.dram_tensor("bn1_gamma", [256], F32, kind="ExternalInput")]
    b_d = [nc.dram_tensor("bn0_beta", [256], F32, kind="ExternalInput"),
           nc.dram_tensor("bn1_beta", [256], F32, kind="ExternalInput")]
    out_d = nc.dram_tensor("out", [IMG, 256, H, H], F32, kind="ExternalOutput")

    xv = x_d.ap().rearrange("n c h w -> c n h w")       # [256, 2, 56, 56]
    ov = out_d.ap().rearrange("n c h w -> c n h w")

    with tile.TileContext(nc) as tc:
        with tc.tile_pool(name="act", bufs=1) as actp, \
             tc.tile_pool(name="wtp", bufs=1) as wtp, \
             tc.tile_pool(name="wq", bufs=4) as wqp, \
             tc.tile_pool(name="qt", bufs=3) as qtp, \
             tc.tile_pool(name="run", bufs=6) as runp, \
             tc.tile_pool(name="st", bufs=1) as stp, \
             tc.tile_pool(name="ps", bufs=8, space="PSUM") as psp, \
             tc.tile_pool(name="dram", bufs=1, space="DRAM") as drp:

            # ---------- static tiles ----------
            # interleaved ci-chunk layout for DoubleRow: [P, ki, rows, col]
            act0 = actp.tile([P, 2, ROWS, H], FP8, name="act0")
            act1 = actp.tile([P, 2, ROWS, H], FP8, name="act1")
            # quantized weight codes, [ci_lo, tap, ki, co] fp8
            wT = [wtp.tile([P, 9, 2, 256], FP8, name=f"w{v}T") for v in range(2)]
            # DRAM spill of conv0 integer outputs (unpadded interior)
            s_dram = [drp.tile([P, IMG, H, H], F32, name=f"s_dram_{c}")
                      for c in range(2)]
            ar_in = [drp.tile([P, 4], F32, name=f"ar_in_{i}") for i in range(2)]
            ar_out = [drp.tile([P, 4], F32, name=f"ar_out_{i}") for i in range(2)]

            # stats / small vectors
            xbn = [stp.tile([P, 16, 6], F32, name=f"xbn_{c}") for c in range(2)]
            sumS = [stp.tile([P, 16], F32, name=f"sumS_{c}") for c in range(2)]
            sqS = [stp.tile([P, 16], F32, name=f"sqS_{c}") for c in range(2)]
            gvec = [[stp.tile([P, 1], F32, name=f"g{v}_{c}") for c in range(2)]
                    for v in range(2)]
            bvec = [[stp.tile([P, 1], F32, name=f"b{v}_{c}") for c in range(2)]
                    for v in range(2)]
            avec = [[stp.tile([P, 1], F32, name=f"a{v}_{c}") for c in range(2)]
                    for v in range(2)]
            bbvec = [[stp.tile([P, 1], F32, name=f"bb{v}_{c}") for c in range(2)]
                     for v in range(2)]
            svec = [stp.tile([P, 1], F32, name=f"scale_{v}") for v in range(2)]
            pk = [stp.tile([P, 4], F32, name=f"pk_{i}") for i in range(2)]
            gpk = [stp.tile([P, 4], F32, name=f"gpk_{i}") for i in range(2)]

            def vtile(name):
                return stp.tile([P, 1], F32, name=name, tag="vtmp", bufs=8)

            # ---------- tiny vector helpers (all on [P,1]) ----------
            def refined_rsqrt(vpe, name):
                """rsqrt(vpe) with 2 Newton refinements (vpe > 0)."""
                r = vtile(f"{name}_r")
                nc.vector.reciprocal(r[:], vpe[:])
                y = vtile(f"{name}_y")
                nc.scalar.activation(y[:], r[:], AF.Sqrt)
                for i in range(2):
                    y2 = vtile(f"{name}_y2{i}")
                    nc.vector.tensor_mul(y2[:], y[:], y[:])
                    t2 = vtile(f"{name}_t2{i}")
                    nc.vector.tensor_mul(t2[:], vpe[:], y2[:])
                    h = vtile(f"{name}_h{i}")
                    nc.vector.tensor_scalar(h[:], t2[:], -0.5, 1.5, OP.mult, OP.add)
                    yn = vtile(f"{name}_yn{i}")
                    nc.vector.tensor_mul(yn[:], y[:], h[:])
                    y = yn
                return y

            def bn_coeffs(v, c, gsum, gsumsq, scale):
                """a,b for z = a*S + b  (= 15 * BN-affine), scale=None for BN0."""
                mean = vtile(f"m{v}{c}")
                nc.vector.tensor_scalar(mean[:], gsum, 1.0 / CNT, None, OP.mult)
                ex2 = vtile(f"e{v}{c}")
                nc.vector.tensor_scalar(ex2[:], gsumsq, 1.0 / CNT, None, OP.mult)
                m2 = vtile(f"m2{v}{c}")
                nc.vector.tensor_mul(m2[:], mean[:], mean[:])
                var = vtile(f"va{v}{c}")
                nc.vector.tensor_sub(var[:], ex2[:], m2[:])
                if scale is not None:
                    s2 = vtile(f"s2{v}{c}")
                    nc.vector.tensor_mul(s2[:], scale[:], scale[:])
                    nc.vector.tensor_mul(var[:], var[:], s2[:])
                    mo = vtile(f"mo{v}{c}")
                    nc.vector.tensor_mul(mo[:], mean[:], scale[:])
                    mean = mo
                vpe = vtile(f"vp{v}{c}")
                nc.vector.tensor_scalar(vpe[:], var[:], EPS, None, OP.add)
                rs = refined_rsqrt(vpe, f"rs{v}{c}")
                grs = vtile(f"gr{v}{c}")
                nc.vector.tensor_mul(grs[:], gvec[v][c][:], rs[:])
                a = avec[v][c]
                if scale is not None:
                    asc = vtile(f"as{v}{c}")
                    nc.vector.tensor_mul(asc[:], grs[:], scale[:])
                    nc.vector.tensor_scalar(a[:], asc[:], 15.0, None, OP.mult)
                else:
                    nc.vector.tensor_scalar(a[:], grs[:], 15.0, None, OP.mult)
                mg = vtile(f"mg{v}{c}")
                nc.vector.tensor_mul(mg[:], mean[:], grs[:])
                mg15 = vtile(f"mh{v}{c}")
                nc.vector.tensor_scalar(mg15[:], mg[:], 15.0, None, OP.mult)
                b15 = vtile(f"bh{v}{c}")
                nc.vector.tensor_scalar(b15[:], bvec[v][c][:], 15.0, None, OP.mult)
                nc.vector.tensor_sub(bbvec[v][c][:], b15[:], mg15[:])

            # ---------- load BN params ----------
            for v in range(2):
                for c in range(2):
                    nc.gpsimd.dma_start(gvec[v][c][:], g_d[v].ap()[c * P:(c + 1) * P])
                    nc.gpsimd.dma_start(bvec[v][c][:], b_d[v].ap()[c * P:(c + 1) * P])

            # ---------- act pad-row zeroing ----------
            with nc.named_scope("memset"):
                for t in (act0, act1):
                    for k in range(2):
                        for r in (0, 57, 58, 115):
                            nc.gpsimd.memset(t[:, k, r, :], 0.0)

            # ---------- weight quantization ----------
            # DRAM layout [i, kh, kw, o] -> contiguous [ci_lo, tap, ki, co]
            def wquant(v):
                mxp = stp.tile([P, 4], F32, name=f"mxp_{v}")
                wv = w_d[v].ap().rearrange("i kh kw o -> i (kh kw) o")
                wnat = {}
                for ki in range(2):
                    for hh in range(2):  # tap halves: 0 -> taps 0..3, 1 -> 4..8
                        t0, t1 = (0, 4) if hh == 0 else (4, 9)
                        wn = wqp.tile([P, t1 - t0, 256], F32,
                                      name=f"wn{v}{ki}{hh}", tag="wnat", bufs=4,
                                      padded_shape=[P, 5, 256])
                        nc.sync.dma_start(
                            wn[:], wv[ki * P:(ki + 1) * P, t0:t1, :])
                        wnat[(ki, hh)] = wn
                for i, (ki, hh) in enumerate(((0, 0), (0, 1), (1, 0), (1, 1))):
                    wn = wnat[(ki, hh)]
                    t = wqp.tile(list(wn.shape), F32, name=f"t{v}{ki}{hh}",
                                 tag="tanh", bufs=4, padded_shape=[P, 5, 256])
                    tf = t.rearrange("p a b -> p (a b)")
                    wf = wn.rearrange("p a b -> p (a b)")
                    nc.scalar.activation(tf[:], wf[:], AF.Tanh)
                    nc.vector.tensor_reduce(
                        mxp[:, i:i + 1], tf[:], AX.X, OP.max,
                        apply_absolute_value=True)
                    wnat[(ki, hh, "t")] = t
                mx1 = vtile(f"mx1_{v}")
                nc.vector.tensor_reduce(mx1[:], mxp[:], AX.X, OP.max,
                                        apply_absolute_value=True)
                msc = stp.tile([1, 1], F32, name=f"msc_{v}")
                nc.gpsimd.tensor_reduce(msc[:], mx1[:], AX.C, OP.max)
                mvec = vtile(f"mvec_{v}")
                nc.gpsimd.partition_broadcast(mvec[:], msc[:])
                # svec = M/225 (psum scale); r = 7.5/M for codes
                nc.vector.tensor_scalar(svec[v][:], mvec[:], 1.0 / 225.0,
                                        None, OP.mult)
                r = vtile(f"rin_{v}")
                nc.vector.reciprocal(r[:], mvec[:])
                for i in range(2):  # Newton: r = r*(2 - M*r)
                    t1_ = vtile(f"rn1_{v}{i}")
                    nc.vector.tensor_mul(t1_[:], mvec[:], r[:])
                    t2_ = vtile(f"rn2_{v}{i}")
                    nc.vector.tensor_scalar(t2_[:], t1_[:], -1.0, 2.0,
                                            OP.mult, OP.add)
                    rn = vtile(f"rn3_{v}{i}")
                    nc.vector.tensor_mul(rn[:], r[:], t2_[:])
                    r = rn
                sc = vtile(f"sc_{v}")
                nc.vector.tensor_scalar(sc[:], r[:], 7.5, None, OP.mult)
                for i, (ki, hh) in enumerate(((0, 0), (0, 1), (1, 0), (1, 1))):
                    t = wnat[(ki, hh, "t")]
                    sh = list(t.shape)
                    tf = t.rearrange("p a b -> p (a b)")
                    z = wqp.tile(sh, F32, name=f"z{v}{ki}{hh}", tag="wz",
                                 bufs=4, padded_shape=[P, 5, 256])
                    zf = z.rearrange("p a b -> p (a b)")
                    nc.vector.tensor_scalar(zf[:], tf[:], sc[:], 7.5,
                                            OP.mult, OP.add)
                    ri = wqp.tile(sh, I8, name=f"ri{v}{ki}{hh}", tag="wr",
                                  bufs=4, padded_shape=[P, 5, 256])
                    rf = ri.rearrange("p a b -> p (a b)")
                    nc.vector.tensor_scalar(rf[:], zf[:], 0.0, 15.0,
                                            OP.max, OP.min)
                    t0 = 0 if hh == 0 else 4
                    dst = wT[v][:, t0:t0 + sh[1], ki, :]
                    nc.vector.tensor_scalar(dst, ri[:], 2.0, -15.0,
                                            OP.mult, OP.add)

            # ---------- BN0 stats over x (streamed) ----------
            with nc.named_scope("stats0"):
                for c in range(2):
                    for img in range(IMG):
                        for blk in range(2):
                            h0 = blk * 28
                            xt = qtp.tile([P, 28, H], F32, name=f"sx{c}{img}{blk}",
                                          tag="qin", bufs=4)
                            nc.sync.dma_start(
                                xt[:], xv[c * P:(c + 1) * P, img, h0:h0 + 28, :])
                            fl = xt.rearrange("p a b -> p (a b)")
                            k = (img * 2 + blk) * 4
                            for cc in range(4):
                                nc.vector.bn_stats(
                                    xbn[c][:, k + cc, :],
                                    fl[:, cc * 392:(cc + 1) * 392])
                    mv = stp.tile([P, 2], F32, name=f"mv0_{c}")
                    nc.vector.bn_aggr(mv[:], xbn[c][:, 0:16, :])
                    # local sums: n*(mean, var+mean^2), n = 6272
                    nc.vector.tensor_scalar(pk[0][:, 2 * c:2 * c + 1],
                                            mv[:, 0:1], 6272.0, None, OP.mult)
                    m2 = vtile(f"xm2_{c}")
                    nc.vector.tensor_mul(m2[:], mv[:, 0:1], mv[:, 0:1])
                    vp = vtile(f"xvp_{c}")
                    nc.vector.tensor_add(vp[:], mv[:, 1:2], m2[:])
                    nc.vector.tensor_scalar(pk[0][:, 2 * c + 1:2 * c + 2],
                                            vp[:], 6272.0, None, OP.mult)
                nc.sync.dma_start(ar_in[0][:], pk[0][:])
                nc.gpsimd.collective_compute(
                    "AllReduce", OP.add, replica_groups=[list(range(N_CORES))],
                    ins=[ar_in[0].opt()], outs=[ar_out[0].opt()])
                nc.sync.dma_start(gpk[0][:], ar_out[0][:])
                for c in range(2):
                    bn_coeffs(0, c, gpk[0][:, 2 * c:2 * c + 1],
                              gpk[0][:, 2 * c + 1:2 * c + 2], None)

            with nc.named_scope("wquant0"):
                wquant(0)

            # ---------- quantize: z = a*in + b (ACT), clip->int8, ->fp8 ----------
            def quantize_block(src_ap, dst_ap, a, b, names):
                z = qtp.tile([P, 28 * H], F32, name=names + "z", tag="qz", bufs=3)
                nc.scalar.activation(z[:], src_ap, AF.Identity,
                                     bias=b[:], scale=a[:])
                u = qtp.tile([P, 28 * H], I8, name=names + "u", tag="qu", bufs=3)
                nc.vector.tensor_scalar(u[:], z[:], 0.0, 15.0, OP.max, OP.min)
                nc.vector.tensor_copy(dst_ap, u[:])

            def act_dst(t, c, lr, n):
                af = t.rearrange("p k r c -> p (k r c)")
                o = (c * ROWS + lr) * H
                return af[:, o:o + n * H]

            with nc.named_scope("quant0"):
                for img in range(IMG):
                    for blk in range(2):
                        for c in range(2):
                            h0 = blk * 28
                            xt = qtp.tile([P, 28, H], F32, name=f"qx{c}{img}{blk}",
                                          tag="qin", bufs=4)
                            nc.sync.dma_start(
                                xt[:], xv[c * P:(c + 1) * P, img, h0:h0 + 28, :])
                            lr = img * 58 + 1 + h0   # logical row
                            quantize_block(xt.rearrange("p a b -> p (a b)")[:],
                                           act_dst(act0, c, lr, 28),
                                           avec[0][c], bbvec[0][c],
                                           f"q0_{c}{img}{blk}")

            # ---------- conv (shared), fp8 DoubleRow, K=256 per matmul ----------
            def conv(v, act, epilogue):
                for gi, grp in enumerate(GROUPS):
                    for co in range(2):
                        psums = []
                        for wi, (r0, nr) in enumerate(grp):
                            ps = psp.tile([P, nr, H], F32,
                                          name=f"ps{v}_{gi}_{co}_{wi}",
                                          tag="psw", padded_shape=[P, 9, H])
                            psums.append(ps)
                        for ti, (dy, dx) in enumerate(TAPS):
                            tap = dy * 3 + dx
                            wlo, whi = max(0, 1 - dx), min(H, H + 1 - dx)
                            jlo = max(0, dx - 1)
                            lhsT = wT[v][:, tap, :, co * P:(co + 1) * P]
                            first = ti == 0
                            last = ti == 8
                            for wi, (r0, nr) in enumerate(grp):
                                rows = slice(r0 + dy - 1, r0 + dy - 1 + nr)
                                if dx == 1:
                                    rhs = act[:, :, rows, :].rearrange(
                                        "p k r c -> p k (r c)")
                                    out = psums[wi].rearrange(
                                        "p r c -> p (r c)")[:, 0:nr * H]
                                else:
                                    rhs = act[:, :, rows, jlo:jlo + whi - wlo]
                                    out = psums[wi][:, :, wlo:whi]
                                nc.tensor.matmul(out, lhsT, rhs,
                                                 start=first, stop=last,
                                                 perf_mode=PM.DoubleRow)
                        for wi, (r0, nr) in enumerate(grp):
                            epilogue(co, r0, nr, psums[wi])

            # ---------- conv0 + epilogue (spill S + interior sums) ----------
            slot_idx = [0, 0]

            def epi0(co, r0, nr, ps):
                psf = ps.rearrange("p r c -> p (r c)")
                for (rl, n, img, h0) in _runs(r0, nr):
                    sl = psf[:, (rl - r0) * H:(rl - r0 + n) * H]
                    tmp = runp.tile([P, n * H], F32, name=f"s_{co}_{rl}",
                                    tag="srun", bufs=6, padded_shape=[P, 9 * H])
                    k = slot_idx[co]
                    slot_idx[co] += 1
                    nc.scalar.activation(tmp[:], sl, AF.Identity,
                                         accum_out=sumS[co][:, k:k + 1])
                    sq = runp.tile([P, n * H], F32, name=f"sq_{co}_{rl}",
                                   tag="sq", bufs=4, padded_shape=[P, 9 * H])
                    nc.vector.scalar_tensor_tensor(
                        sq[:], tmp[:], 0.0, tmp[:], OP.bypass, OP.mult,
                        accum_out=sqS[co][:, k:k + 1])
                    nc.sync.dma_start(s_dram[co][:, img, h0:h0 + n, :], tmp[:])

            with nc.named_scope("conv0"):
                conv(0, act0, epi0)

            with nc.named_scope("wquant1"):
                wquant(1)

            # ---------- BN1 stats + AR + coeffs ----------
            with nc.named_scope("stats1"):
                for c in range(2):
                    ns = slot_idx[c]
                    nc.vector.tensor_reduce(pk[1][:, 2 * c:2 * c + 1],
                                            sumS[c][:, 0:ns], AX.X, OP.add)
                    nc.vector.tensor_reduce(pk[1][:, 2 * c + 1:2 * c + 2],
                                            sqS[c][:, 0:ns], AX.X, OP.add)
                nc.sync.dma_start(ar_in[1][:], pk[1][:])
                nc.gpsimd.collective_compute(
                    "AllReduce", OP.add, replica_groups=[list(range(N_CORES))],
                    ins=[ar_in[1].opt()], outs=[ar_out[1].opt()])
                nc.sync.dma_start(gpk[1][:], ar_out[1][:])
                for c in range(2):
                    bn_coeffs(1, c, gpk[1][:, 2 * c:2 * c + 1],
                              gpk[1][:, 2 * c + 1:2 * c + 2], svec[0])

            # ---------- quantize1: S -> act1 codes ----------
            with nc.named_scope("quant1"):
                for img in range(IMG):
                    for blk in range(2):
                        for c in range(2):
                            h0 = blk * 28
                            st = qtp.tile([P, 28, H], F32, name=f"qs{c}{img}{blk}",
                                          tag="qin", bufs=4)
                            nc.scalar.dma_start(
                                st[:], s_dram[c][:, img, h0:h0 + 28, :])
                            lr = img * 58 + 1 + h0
                            quantize_block(st.rearrange("p a b -> p (a b)")[:],
                                           act_dst(act1, c, lr, 28),
                                           avec[1][c], bbvec[1][c],
                                           f"q1_{c}{img}{blk}")

            # ---------- conv1 + residual epilogue ----------
            def epi1(co, r0, nr, ps):
                psf = ps.rearrange("p r c -> p (r c)")
                for (rl, n, img, h0) in _runs(r0, nr):
                    sl = psf[:, (rl - r0) * H:(rl - r0 + n) * H]
                    xt = runp.tile([P, n * H], F32, name=f"x_{co}_{rl}",
                                   tag="xrun", bufs=6, padded_shape=[P, 9 * H])
                    nc.sync.dma_start(
                        xt[:], xv[co * P:(co + 1) * P, img, h0:h0 + n, :])
                    ot = runp.tile([P, n * H], F32, name=f"o_{co}_{rl}",
                                   tag="srun", bufs=6, padded_shape=[P, 9 * H])
                    nc.vector.scalar_tensor_tensor(
                        ot[:], sl, svec[1][:], xt[:], OP.mult, OP.add)
                    nc.sync.dma_start(
                        ov[co * P:(co + 1) * P, img, h0:h0 + n, :],
                        ot.rearrange("p (a b) -> p a b", b=H)[:])

            with nc.named_scope("conv1"):
                conv(1, act1, epi1)

    nc.compile()
    return nc


def _install_ntff_hook():
    """Provide antenv.axon_hooks (absent in this image) via ctypes so that
    run_bass_kernel_spmd(trace=True) can capture NTFF profiles."""
    try:
        from antenv.axon_hooks import get_axon_ntff_profile_hook  # noqa: F401
        return
    except ImportError:
        pass
    import contextlib
    import ctypes
    import types

    so_path = "/opt/axon/libaxon_pjrt.so"
    if not os.path.exists(so_path):
        return
    lib = ctypes.CDLL(so_path)
    if not hasattr(lib, "axon_start_nrt_profile"):
        return
    lib.axon_start_nrt_profile.argtypes = [ctypes.POINTER(ctypes.c_int64),
                                           ctypes.c_size_t]
    lib.axon_start_nrt_profile.restype = ctypes.c_int64
    lib.axon_stop_nrt_profile.argtypes = [ctypes.c_char_p]
    lib.axon_stop_nrt_profile.restype = ctypes.c_int64

    @contextlib.contextmanager
    def _hook(output_dir, device_ids):
        import jax
        jax.devices()
        if device_ids:
            ids = (ctypes.c_int64 * len(device_ids))(*device_ids)
            rc = lib.axon_start_nrt_profile(ids, len(device_ids))
        else:
            rc = lib.axon_start_nrt_profile(None, 0)
        if rc != 0:
            raise RuntimeError(f"axon_start_nrt_profile rc={rc}")
        try:
            yield
        finally:
            n = lib.axon_stop_nrt_profile(str(output_dir).encode())
            print(f"ntff profile: {n} file(s) written to {output_dir}")

    hook_holder = [_hook]
    mod = types.ModuleType("antenv.axon_hooks")
    mod.get_axon_ntff_profile_hook = lambda: hook_holder[0]
    mod.set_axon_ntff_profile_hook = lambda h: hook_holder.__setitem__(0, h)
    import antenv
    sys.modules["antenv.axon_hooks"] = mod
    antenv.axon_hooks = mod


_NC = None


def _get_nc():
    global _NC
    if _NC is None:
        _NC = build()
    return _NC


LAST_RESULTS = None


def kernel(x, bn0_gamma, bn0_beta, conv0_w, bn1_gamma, bn1_beta, conv1_w):
    global LAST_RESULTS
    nc = _get_nc()
    shared = {
        # permute OIHW -> [i, kh, kw, o] so on-chip weight access is contiguous
        "conv0_w": np.ascontiguousarray(
            np.asarray(conv0_w, np.float32).transpose(1, 2, 3, 0)),
        "conv1_w": np.ascontiguousarray(
            np.asarray(conv1_w, np.float32).transpose(1, 2, 3, 0)),
        "bn0_gamma": np.ascontiguousarray(bn0_gamma, np.float32),
        "bn0_beta": np.ascontiguousarray(bn0_beta, np.float32),
        "bn1_gamma": np.ascontiguousarray(bn1_gamma, np.float32),
        "bn1_beta": np.ascontiguousarray(bn1_beta, np.float32),
    }
    x = np.ascontiguousarray(x, np.float32)
    in_maps = [{"x": x[2 * c:2 * c + 2], **shared} for c in range(N_CORES)]
    trace = bool(int(os.environ.get("KERNEL_TRACE", "0")))
    if trace:
        _install_ntff_hook()
    res = bass_utils.run_bass_kernel_spmd(
        nc, in_maps, core_ids=list(range(N_CORES)), trace=trace)
    LAST_RESULTS = res
    return np.concatenate([res.results[c]["out"] for c in range(N_CORES)], axis=0)
